# revision 22
# baseline (speedup 1.0000x reference)
"""Trainium2 Bass kernel for nn_DeepMapping2D (histogram_binning).

Reference semantics: per cloud, quantize points to integer mm bins
(q = round_half_even(1000*p)), histogram into a 1024x1024 grid (shifted by
per-cloud coordinate minima), threshold counts (count/N > 2e-4 <=> count>=53),
sort the 0/1 occupancy descending, truncate to TOPK.  The sorted vector is K
ones then zeros, K = #bins with count >= 53.  Shifting by the minima is a
bijection on occupied bins, so K is shift-invariant and the device can work
on unshifted bin ids s = qx*1024 + qz (fine id, < 2^20).

Device algorithm (exact, two launches, all histogram work on device):

Phase 1 (screen): per cloud, the exact 2^14-bin coarse histogram H14 over
c14 = s>>6, computed as a PSUM-matmul scatter: per column of 128 points,
build 128-wide one-hots of hi7=c14>>7 and lo7=c14&127 by comparing a
constant iota row against the point's value (DVE tensor_scalar is_equal with
a per-partition scalar), then accumulate onehot_hi^T @ onehot_lo into PSUM
(bf16 0/1 inputs are exact; fp32 accumulation).  Only the thresholded int8
mask (H14 >= 53) goes back to the host.

Host: candidate cells = {c14 : H14[c14] >= 53} (every fine bin with count
>= 53 lives in one, since H14 upper-bounds its 64 fine bins).  ~1.4k/cloud
for the rbg-generated inputs.  Sorted, padded with -1 to NCHUNK*128 int16.

Phase 2 (refine): per cloud, exact fine counts for every candidate cell:
per column, one membership one-hot against the candidate row (int16
candidates streamed at DVE 4x, compared against the point's c14 as the
per-partition scalar) and one 64-wide one-hot of low6 = s&63; NCHUNK
matmuls accumulate membership^T @ onehot_low6 into PSUM -> exact
[candidate, low6] fine counts.  Threshold >= 53, count via ones^T @ mask
matmul, giving K per cloud.  The host formats the final rows (K ones then
zeros) from the device-computed K values.

Transport: the axon PJRT tunnel moves ~85 MB/s with ~75 ms per-call fixed
cost (a single sharded device_put beats any chunked/parallel scheme), so
the wall clock is dominated by host->device bytes.  The host pre-rounds
coordinates (bit-exact: f32 mult by 1000 + rint == the device's
+/-1.5*2^23 trick) and packs each point into 2.5 bytes: the low 8 bits of
qx and qz as an interleaved u8 plane, and the high 2 bits of each packed
as one nibble per point, two points per byte.  That is 40 MB instead of
the 128 MB raw f32 input; the device unpacks with exact f32 floor tricks
(a dozen cheap DVE ops per tile).  The upload happens ONCE: the packed q
lives on device as a committed sharded jax array reused by both phase
launches.  Both phases run through persistent jit(shard_map(bass_exec))
wrappers built once per process, so warm calls skip retrace/relower
entirely.

Host guards keep the kernel exact for arbitrary inputs: clouds whose
candidate count exceeds capacity fall back to an exact numpy recomputation
of that cloud.

Sharding: data-parallel over batch: 64 clouds -> 8 cores x 8 clouds.
"""

import numpy as np

B = 64
N = 262144
TOPK = 5120
NCORES = 8
CLOUDS_PER_CORE = B // NCORES
P = 128
GZ = 1024
NCHUNK = 12  # candidate capacity = NCHUNK*128 cells per cloud
CAND_CAP = NCHUNK * P
THRESH_COUNT = 53.0
C23 = 12582912.0  # 1.5 * 2^23

_cache = {}


def _chain(nc, tc, pools, qpk, c, F, mybir, need_low6, col0=0, n_points=N):
    """Elementwise chain for columns [col0, col0+F) of cloud c.

    qpk is the packed 2.5-byte/point format: per cloud, 2*n_points u8 of
    interleaved (qx&255, qz&255), then n_points/2 u8 of hi nibbles
    (qx>>8)*4+(qz>>8), two points per byte.  Computes s = qx*1024 + qz,
    c14 = s>>6 and optionally low6 = s&63, all exact in f32.
    """
    import concourse.bass as bass

    f32 = mybir.dt.float32
    u8 = mybir.dt.uint8
    op = mybir.AluOpType
    workp, chainp = pools
    FT = n_points // P  # full points-per-partition (column count)

    lo_src = qpk[c][: 2 * n_points].rearrange("(p x) -> p x", p=P)
    hp_src = qpk[c][2 * n_points :].rearrange("(p x) -> p x", p=P)
    tin = workp.tile([P, 2 * F], u8, tag="tin")
    nc.gpsimd.dma_start(out=tin[:], in_=lo_src[:, 2 * col0 : 2 * (col0 + F)])
    thp = workp.tile([P, F // 2], u8, tag="thp")
    nc.gpsimd.dma_start(out=thp[:], in_=hp_src[:, col0 // 2 : (col0 + F) // 2])

    # unpack hi nibbles: he = thp>>4, ho = thp&15 (floor via the mult +
    # offset + C23 round trick, all exact in f32), interleave to [P, F]
    t1 = chainp.tile([P, F // 2], f32, tag="t1")
    nc.vector.tensor_scalar(
        out=t1[:], in0=thp[:], scalar1=0.0625, scalar2=0.46875,
        op0=op.mult, op1=op.subtract,
    )
    he = chainp.tile([P, F // 2], f32, tag="he")
    nc.vector.tensor_scalar(
        out=he[:], in0=t1[:], scalar1=C23, scalar2=C23, op0=op.add, op1=op.subtract
    )
    ho = chainp.tile([P, F // 2], f32, tag="ho")
    nc.vector.scalar_tensor_tensor(
        out=ho[:], in0=he[:], scalar=-16.0, in1=thp[:], op0=op.mult, op1=op.add
    )
    hfull = chainp.tile([P, F], f32, tag="hfull")
    hv = hfull[:].rearrange("p (f two) -> p two f", two=2)
    nc.vector.tensor_copy(out=hv[:, 0], in_=he[:])
    nc.vector.tensor_copy(out=hv[:, 1], in_=ho[:])
    # hx = h>>2, hz = h&3
    t2 = chainp.tile([P, F], f32, tag="t2")
    nc.vector.tensor_scalar(
        out=t2[:], in0=hfull[:], scalar1=0.25, scalar2=0.375,
        op0=op.mult, op1=op.subtract,
    )
    hx = chainp.tile([P, F], f32, tag="hx")
    nc.vector.tensor_scalar(
        out=hx[:], in0=t2[:], scalar1=C23, scalar2=C23, op0=op.add, op1=op.subtract
    )
    hz = chainp.tile([P, F], f32, tag="hz")
    nc.vector.scalar_tensor_tensor(
        out=hz[:], in0=hx[:], scalar=-4.0, in1=hfull[:], op0=op.mult, op1=op.add
    )
    # qx = lox + 256*hx, qz = loz + 256*hz, s = qx*1024 + qz (exact, < 2^24)
    tv = tin[:].rearrange("p (f t) -> p t f", t=2)
    qx = chainp.tile([P, F], f32, tag="qx")
    nc.vector.scalar_tensor_tensor(
        out=qx[:], in0=hx[:], scalar=256.0, in1=tv[:, 0], op0=op.mult, op1=op.add
    )
    qz = chainp.tile([P, F], f32, tag="qz")
    nc.vector.scalar_tensor_tensor(
        out=qz[:], in0=hz[:], scalar=256.0, in1=tv[:, 1], op0=op.mult, op1=op.add
    )
    ts_ = chainp.tile([P, F], f32, tag="ts")
    nc.vector.scalar_tensor_tensor(
        out=ts_[:], in0=qx[:], scalar=1024.0, in1=qz[:], op0=op.mult, op1=op.add
    )
    # c14 = floor(s/64): s*2^-6 is exact, offset by -63/128 (exact), then the
    # fused (+C23, -C23) forces a round-to-nearest at integer granularity.
    tu = chainp.tile([P, F], f32, tag="tu")
    nc.vector.tensor_scalar(
        out=tu[:], in0=ts_[:], scalar1=0.015625, scalar2=0.4921875,
        op0=op.mult, op1=op.subtract,
    )
    tc14 = workp.tile([P, F], f32, tag="tc14")
    nc.vector.tensor_scalar(
        out=tc14[:], in0=tu[:], scalar1=C23, scalar2=C23, op0=op.add, op1=op.subtract
    )
    tlow6 = None
    if need_low6:
        # low6 = s - 64*c14
        tlow6 = workp.tile([P, F], f32, tag="tlow6")
        nc.vector.scalar_tensor_tensor(
            out=tlow6[:], in0=tc14[:], scalar=-64.0, in1=ts_[:],
            op0=op.mult, op1=op.add,
        )
    return tc14, tlow6, ts_


def build_phase1(n_clouds=CLOUDS_PER_CORE, n_points=N, nchunk=NCHUNK, unroll=32):
    """Per-cloud exact 2^14-bin coarse histogram -> threshold -> on-device
    compaction of candidate cell ids (sorted by cell id, -1 padded) plus the
    true candidate count.  candv stays in device DRAM for phase 2.

    Compaction: maskf[h,l] = [hist >= 53]; rc[h] = row count; prefix[h] =
    strict lower-triangular matmul over rc; off[h,l] = in-row exclusive
    prefix (log-shift scan); slot = prefix + off (position of cell (h,l) in
    the sorted candidate list).  For each l, a one-hot of slot against a
    0..CAP-1 iota row turns into two PSUM matmuls accumulating 128*h and l
    into candv[slot]; unused slots get -1 via the total count."""
    import concourse.bass as bass
    import concourse.mybir as mybir
    from concourse.tile import TileContext

    f32, bf16 = mybir.dt.float32, mybir.dt.bfloat16
    i16, i32 = mybir.dt.int16, mybir.dt.int32
    op = mybir.AluOpType
    F = n_points // P
    cap = nchunk * P

    from concourse import bacc

    nc = bacc.Bacc("TRN2", target_bir_lowering=False, debug=False)
    qpk = nc.declare_dram_parameter(
        "qpk", [n_clouds, 2 * n_points + n_points // 2], mybir.dt.uint8, isOutput=False
    )
    candv = nc.declare_dram_parameter("candv", [n_clouds, cap], i16, isOutput=True)
    ccount = nc.declare_dram_parameter("ccount", [1, n_clouds], f32, isOutput=True)
    pscr = nc.dram_tensor("pscr", [P], f32)

    with TileContext(nc) as tc:
        with (
            tc.tile_pool(name="const", bufs=1) as constp,
            tc.tile_pool(name="work", bufs=2) as workp,
            tc.tile_pool(name="chain", bufs=1) as chainp,
            tc.tile_pool(name="hilo", bufs=2) as hilop,
            tc.tile_pool(name="oh", bufs=8) as ohp,
            tc.tile_pool(name="cmp", bufs=2) as cmpp,
            tc.tile_pool(name="sel", bufs=4) as selp,
            tc.tile_pool(name="psum", bufs=2, space="PSUM") as psump,
            tc.tile_pool(name="cps", bufs=1, space="PSUM") as cpsp,
        ):
            iota_i = constp.tile([P, P], i32)
            nc.gpsimd.iota(iota_i[:], pattern=[[1, P]], base=0, channel_multiplier=0)
            iota_bf = constp.tile([P, P], bf16)
            nc.vector.tensor_copy(out=iota_bf[:], in_=iota_i[:])
            iota_f = constp.tile([P, P], f32)
            nc.vector.tensor_copy(out=iota_f[:], in_=iota_i[:])
            iotacap_i = constp.tile([P, cap], i32)
            nc.gpsimd.iota(iotacap_i[:], pattern=[[1, cap]], base=0, channel_multiplier=0)
            iotacap_f = constp.tile([P, cap], f32)
            nc.vector.tensor_copy(out=iotacap_f[:], in_=iotacap_i[:])
            ones_bf = constp.tile([P, 1], bf16)
            nc.vector.memset(ones_bf[:], 1.0)
            # per-partition index column via a DRAM roundtrip of an iota row
            nc.gpsimd.dma_start(out=pscr[:], in_=iota_f[0:1, :])
            piota_f = constp.tile([P, 1], f32)
            nc.gpsimd.dma_start(out=piota_f[:], in_=pscr[:].rearrange("(b o) -> b o", o=1))
            p128_bf = constp.tile([P, 1], bf16)
            nc.vector.tensor_scalar(
                out=p128_bf[:], in0=piota_f[:], scalar1=128.0, scalar2=None, op0=op.mult
            )
            # LT[k,h] = 1 if k < h (strict lower-triangular, as lhsT)
            lt_bf = constp.tile([P, P], bf16)
            nc.vector.tensor_scalar(
                out=lt_bf[:], in0=iota_bf[:], scalar1=piota_f[:, 0:1], scalar2=None,
                op0=op.is_gt,
            )
            cc_sb = constp.tile([1, n_clouds], f32)

            FC = min(512, F)  # chain chunk width (columns)
            for c in range(n_clouds):
                thi = hilop.tile([P, F], f32, tag="thi")
                tlo = hilop.tile([P, F], f32, tag="tlo")
                for col0 in range(0, F, FC):
                    tc14, _, _ = _chain(
                        nc, tc, (workp, chainp), qpk, c, FC, mybir,
                        need_low6=False, col0=col0, n_points=n_points,
                    )
                    # hi7 = floor(c14/128); lo7 = c14 - 128*hi7
                    thif = chainp.tile([P, FC], f32, tag="thif")
                    nc.vector.tensor_scalar(
                        out=thif[:], in0=tc14[:], scalar1=0.0078125,
                        scalar2=0.49609375, op0=op.mult, op1=op.subtract,
                    )
                    sl = slice(col0, col0 + FC)
                    nc.vector.tensor_scalar(
                        out=thi[:, sl], in0=thif[:], scalar1=C23, scalar2=C23,
                        op0=op.add, op1=op.subtract,
                    )
                    nc.vector.scalar_tensor_tensor(
                        out=tlo[:, sl], in0=thi[:, sl], scalar=-128.0,
                        in1=tc14[:], op0=op.mult, op1=op.add,
                    )
                hist = psump.tile([P, P], f32, tag="hist")
                nc.vector.memset(hist[:], 0.0)

                def body(iv, thi=thi, tlo=tlo, hist=hist):
                    ohh = ohp.tile([P, P], bf16, tag="ohh")
                    ohl = ohp.tile([P, P], bf16, tag="ohl")
                    nc.vector.tensor_scalar(
                        out=ohh[:], in0=iota_bf[:],
                        scalar1=thi[:, bass.ds(iv, 1)], scalar2=None,
                        op0=op.is_equal,
                    )
                    nc.vector.tensor_scalar(
                        out=ohl[:], in0=iota_bf[:],
                        scalar1=tlo[:, bass.ds(iv, 1)], scalar2=None,
                        op0=op.is_equal,
                    )
                    nc.tensor.matmul(
                        out=hist[:], lhsT=ohh[:], rhs=ohl[:],
                        start=False, stop=True, skip_group_check=True,
                    )

                tc.For_i_unrolled(0, F, 1, body, max_unroll=unroll)

                # ---- threshold + on-device candidate compaction ----
                maskf = cmpp.tile([P, P], f32, tag="maskf")
                nc.vector.tensor_scalar(
                    out=maskf[:], in0=hist[:], scalar1=THRESH_COUNT - 0.5,
                    scalar2=None, op0=op.is_ge,
                )
                rc = cmpp.tile([P, 1], f32, tag="rc")
                nc.vector.tensor_reduce(
                    out=rc[:], in_=maskf[:], axis=mybir.AxisListType.X, op=op.add
                )
                rc_bf = cmpp.tile([P, 1], bf16, tag="rcbf")
                nc.vector.tensor_copy(out=rc_bf[:], in_=rc[:])
                # prefix[h] = sum_{k<h} rc[k]; total = sum_k rc[k]
                pre_ps = cpsp.tile([P, 1], f32, tag="preps")
                nc.tensor.matmul(
                    out=pre_ps[:], lhsT=lt_bf[:], rhs=rc_bf[:], start=True, stop=True
                )
                tot_ps = cpsp.tile([1, 1], f32, tag="totps")
                nc.tensor.matmul(
                    out=tot_ps[:], lhsT=ones_bf[:], rhs=rc_bf[:], start=True, stop=True
                )
                pre_sb = cmpp.tile([P, 1], f32, tag="presb")
                nc.vector.tensor_copy(out=pre_sb[:], in_=pre_ps[:])
                tot_sb = cmpp.tile([1, 1], f32, tag="totsb")
                nc.vector.tensor_copy(out=tot_sb[:], in_=tot_ps[:])
                nc.vector.tensor_copy(out=cc_sb[0:1, c : c + 1], in_=tot_sb[:])
                # in-row inclusive scan (log shifts, ping-pong), then exclusive
                sA = cmpp.tile([P, P], f32, tag="scanA")
                sB = cmpp.tile([P, P], f32, tag="scanB")
                cur, nxt = sA, sB
                nc.vector.tensor_copy(out=cur[:], in_=maskf[:])
                for sh in (1, 2, 4, 8, 16, 32, 64):
                    nc.vector.tensor_copy(out=nxt[:, :sh], in_=cur[:, :sh])
                    nc.vector.tensor_tensor(
                        out=nxt[:, sh:], in0=cur[:, sh:], in1=cur[:, : P - sh],
                        op=op.add,
                    )
                    cur, nxt = nxt, cur
                # slot = prefix + (incl - mask); park non-candidates at 20000
                excl = cmpp.tile([P, P], f32, tag="excl")
                nc.vector.tensor_tensor(
                    out=excl[:], in0=cur[:], in1=maskf[:], op=op.subtract
                )
                slotA = cmpp.tile([P, P], f32, tag="slotA")
                nc.vector.tensor_scalar(
                    out=slotA[:], in0=excl[:], scalar1=pre_sb[:, 0:1],
                    scalar2=20000.0, op0=op.add, op1=op.subtract,
                )
                slotB = cmpp.tile([P, P], f32, tag="slotB")
                nc.vector.tensor_tensor(
                    out=slotB[:], in0=slotA[:], in1=maskf[:], op=op.mult
                )
                slot = cmpp.tile([P, P], f32, tag="slot")
                nc.vector.tensor_scalar(
                    out=slot[:], in0=slotB[:], scalar1=20000.0, scalar2=None, op0=op.add
                )
                # scatter cell ids to slots: candv[slot(h,l)] = 128*h + l
                cv_ps = cpsp.tile([1, cap], f32, tag="cvps")
                nc.vector.memset(cv_ps[:], 0.0)
                for l in range(P):
                    sel = selp.tile([P, cap], bf16, tag="sel")
                    nc.vector.tensor_scalar(
                        out=sel[:], in0=iotacap_f[:],
                        scalar1=slot[:, l : l + 1], scalar2=None, op0=op.is_equal,
                    )
                    for g in range(cap // 512):
                        gs = slice(g * 512, (g + 1) * 512)
                        nc.tensor.matmul(
                            out=cv_ps[:, gs], lhsT=p128_bf[:], rhs=sel[:, gs],
                            start=False, stop=True, skip_group_check=True,
                        )
                        nc.tensor.matmul(
                            out=cv_ps[:, gs], lhsT=iota_bf[:, l : l + 1], rhs=sel[:, gs],
                            start=False, stop=True, skip_group_check=True,
                        )
                # unused slots (j >= total) -> -1
                unused = cmpp.tile([1, cap], f32, tag="unused")
                nc.vector.tensor_scalar(
                    out=unused[:], in0=iotacap_f[0:1, :],
                    scalar1=tot_sb[0:1, 0:1], scalar2=None, op0=op.is_ge,
                )
                cfin = cmpp.tile([1, cap], f32, tag="cfin")
                nc.vector.tensor_tensor(
                    out=cfin[:], in0=cv_ps[:], in1=unused[:], op=op.subtract
                )
                cfin_i = cmpp.tile([1, cap], i16, tag="cfini")
                nc.vector.tensor_copy(out=cfin_i[:], in_=cfin[:])
                nc.gpsimd.dma_start(out=candv[c : c + 1, :], in_=cfin_i[:])

            nc.gpsimd.dma_start(out=ccount[:, :], in_=cc_sb[:])
    nc.compile()
    return nc


def build_phase2(n_clouds=CLOUDS_PER_CORE, n_points=N, nchunk=NCHUNK, unroll=16):
    """Exact [candidate,64] fine counts -> K per cloud."""
    import concourse.bass as bass
    import concourse.mybir as mybir
    from concourse.tile import TileContext

    f32, bf16 = mybir.dt.float32, mybir.dt.bfloat16
    i16, i32 = mybir.dt.int16, mybir.dt.int32
    op = mybir.AluOpType
    F = n_points // P
    cap = nchunk * P

    from concourse import bacc

    nc = bacc.Bacc("TRN2", target_bir_lowering=False, debug=False)
    qpk = nc.declare_dram_parameter(
        "qpk", [n_clouds, 2 * n_points + n_points // 2], mybir.dt.uint8, isOutput=False
    )
    cands = nc.declare_dram_parameter("cands", [n_clouds, cap], i16, isOutput=False)
    kvals = nc.declare_dram_parameter("kvals", [1, n_clouds], f32, isOutput=True)

    with TileContext(nc) as tc:
        with (
            tc.tile_pool(name="const", bufs=1) as constp,
            tc.tile_pool(name="work", bufs=2) as workp,
            tc.tile_pool(name="chain", bufs=1) as chainp,
            tc.tile_pool(name="oh", bufs=8) as ohp,
            tc.tile_pool(name="mk", bufs=4) as mkp,
            tc.tile_pool(name="psum", bufs=1, space="PSUM") as psump,
            tc.tile_pool(name="kps", bufs=1, space="PSUM") as kpsp,
        ):
            iota64_i = constp.tile([P, 64], i32)
            nc.gpsimd.iota(iota64_i[:], pattern=[[1, 64]], base=0, channel_multiplier=0)
            iota64_bf = constp.tile([P, 64], bf16)
            nc.vector.tensor_copy(out=iota64_bf[:], in_=iota64_i[:])
            ones_bf = constp.tile([P, 1], bf16)
            nc.vector.memset(ones_bf[:], 1.0)
            kv_sb = constp.tile([1, n_clouds], f32)

            for c in range(n_clouds):
                tc14, tlow6, _ = _chain(
                    nc, tc, (workp, chainp), qpk, c, F, mybir, need_low6=True,
                    n_points=n_points,
                )
                # candidate row broadcast to all partitions
                candbc = workp.tile([P, cap], i16, tag="candbc")
                cand_src = bass.AP(
                    tensor=cands.tensor if hasattr(cands, "tensor") else cands,
                    offset=c * cap,
                    ap=[[0, P], [1, cap]],
                )
                nc.gpsimd.dma_start(out=candbc[:], in_=cand_src)

                hist = psump.tile([P, cap], f32, tag="hist")
                nc.vector.memset(hist[:], 0.0)

                def body(iv):
                    memb = ohp.tile([P, cap], bf16, tag="memb")
                    loh = ohp.tile([P, 64], bf16, tag="loh")
                    nc.vector.tensor_scalar(
                        out=memb[:], in0=candbc[:],
                        scalar1=tc14[:, bass.ds(iv, 1)], scalar2=None,
                        op0=op.is_equal,
                    )
                    nc.vector.tensor_scalar(
                        out=loh[:], in0=iota64_bf[:],
                        scalar1=tlow6[:, bass.ds(iv, 1)], scalar2=None,
                        op0=op.is_equal,
                    )
                    # transposed accumulation: hist[w, cand] += loh^T @ memb,
                    # 512-wide moving slices so the 64-wide stationary loh is
                    # shared and PE streams at full width
                    for g in range(cap // 512):
                        nc.tensor.matmul(
                            out=hist[:64, g * 512 : (g + 1) * 512],
                            lhsT=loh[:],
                            rhs=memb[:, g * 512 : (g + 1) * 512],
                            start=False, stop=True, skip_group_check=True,
                        )

                tc.For_i_unrolled(0, F, 1, body, max_unroll=unroll)

                # K = sum over candidates/low6 of [count >= 53]
                kps = kpsp.tile([1, cap], f32, tag="kps")
                for g in range(cap // 512):
                    mask = mkp.tile([P, 512], bf16, tag="mask")
                    nc.vector.tensor_scalar(
                        out=mask[:64, :], in0=hist[:64, g * 512 : (g + 1) * 512],
                        scalar1=52.5, scalar2=None, op0=op.is_ge,
                    )
                    nc.tensor.matmul(
                        out=kps[:1, g * 512 : (g + 1) * 512],
                        lhsT=ones_bf[:64, :], rhs=mask[:64, :],
                        start=True, stop=True,
                    )
                nc.vector.tensor_reduce(
                    out=kv_sb[:1, c : c + 1], in_=kps[:],
                    axis=mybir.AxisListType.X, op=op.add,
                )

            nc.gpsimd.dma_start(out=kvals[:, :], in_=kv_sb[:])
    nc.compile()
    return nc


def _host_exact(points):
    """Exact numpy replica of the reference for one cloud. [N,2] f32 -> [TOPK]."""
    q = np.round(np.float32(1000.0) * points.astype(np.float32))
    xi = (q[:, 0] - q[:, 0].min()).astype(np.int64)
    zi = (q[:, 1] - q[:, 1].min()).astype(np.int64)
    idx = xi * GZ + zi
    counts = np.bincount(idx, minlength=1024 * GZ).astype(np.float32)
    occ = counts / np.float32(points.shape[0]) > np.float32(0.0002)
    k = min(int(occ.sum()), TOPK)
    out = np.zeros((TOPK,), np.float32)
    out[:k] = 1.0
    return out


def _numba_pack():
    if "nbpack" in _cache:
        return _cache["nbpack"]
    try:
        import numba

        @numba.njit(cache=False)
        def _nb_pack(pcd, out):
            nb, npts = pcd.shape[0], pcd.shape[1]
            for b in range(nb):
                base = 2 * npts
                for j in range(npts // 2):
                    n0 = 2 * j
                    qx0 = np.int32(np.rint(np.float32(1000.0) * pcd[b, n0, 0]))
                    qz0 = np.int32(np.rint(np.float32(1000.0) * pcd[b, n0, 1]))
                    qx1 = np.int32(np.rint(np.float32(1000.0) * pcd[b, n0 + 1, 0]))
                    qz1 = np.int32(np.rint(np.float32(1000.0) * pcd[b, n0 + 1, 1]))
                    out[b, 4 * j] = qx0 & 255
                    out[b, 4 * j + 1] = qz0 & 255
                    out[b, 4 * j + 2] = qx1 & 255
                    out[b, 4 * j + 3] = qz1 & 255
                    nib0 = ((qx0 >> 8) << 2) | (qz0 >> 8)
                    nib1 = ((qx1 >> 8) << 2) | (qz1 >> 8)
                    out[b, base + j] = (nib0 << 4) | nib1

        _cache["nbpack"] = _nb_pack
    except Exception:
        _cache["nbpack"] = None
    return _cache["nbpack"]


def _quantize_pack(pcd):
    """q = round_half_even(1000*pcd), bit-exact vs the reference's jnp.round
    (f32 multiply, then IEEE round-to-nearest-even), packed to 2.5 bytes per
    point: [2N u8 of interleaved low bytes | N/2 u8 of paired hi nibbles]."""
    nb = pcd.shape[0]
    npts = pcd.shape[1]
    out = np.empty((nb, 2 * npts + npts // 2), np.uint8)
    nbp = _numba_pack()
    if nbp is not None:
        nbp(pcd, out)
        return out
    t = np.empty((npts, 2), np.float32)
    for b in range(nb):
        np.multiply(pcd[b], np.float32(1000.0), out=t)
        np.rint(t, out=t)
        qi = t.astype(np.int16)  # [N, 2], values 0..1000
        v = qi.view(np.uint8)  # [N, 4]: qx_lo qx_hi qz_lo qz_hi
        lo = out[b, : 2 * npts].reshape(npts, 2)
        lo[:, 0] = v[:, 0]
        lo[:, 1] = v[:, 2]
        hb = (v[:, 1] << 2) | v[:, 3]
        out[b, 2 * npts :] = (hb[0::2] << 4) | hb[1::2]
    return out


def _make_exec(nc, n_cores, mesh):
    """Persistent jit(shard_map(bass_exec)) wrapper for a compiled Bass
    module: built once, reused every call (C++ fast-path dispatch after the
    first).  Mirrors concourse.bass2jax.run_bass_via_pjrt but accepts
    device-resident jax arrays so large inputs upload only once."""
    import jax
    import concourse.mybir as mybir
    from concourse import bass2jax
    from jax.sharding import PartitionSpec
    from jax.experimental.shard_map import shard_map

    bass2jax.install_neuronx_cc_hook()
    assert nc.dbg_addr is None and not nc.dbg_callbacks

    partition_name = nc.partition_id_tensor.name if nc.partition_id_tensor else None
    in_names, out_names, out_avals = [], [], []
    for alloc in nc.m.functions[0].allocations:
        if not isinstance(alloc, mybir.MemoryLocationSet):
            continue
        name = alloc.memorylocations[0].name
        if alloc.kind == "ExternalInput":
            if name != partition_name:
                in_names.append(name)
        elif alloc.kind == "ExternalOutput":
            out_names.append(name)
            out_avals.append(
                jax.core.ShapedArray(tuple(alloc.tensor_shape), mybir.dt.np(alloc.dtype))
            )
    n_params = len(in_names)
    all_names = in_names + out_names + ([partition_name] if partition_name else [])
    donate = tuple(range(n_params, n_params + len(out_names)))

    def _body(*args):
        operands = list(args)
        if partition_name is not None:
            operands.append(bass2jax.partition_id_tensor())
        return tuple(
            bass2jax._bass_exec_p.bind(
                *operands,
                out_avals=tuple(out_avals),
                in_names=tuple(all_names),
                out_names=tuple(out_names),
                lowering_input_output_aliases=(),
                sim_require_finite=True,
                sim_require_nnan=True,
                nc=nc,
            )
        )

    nio = n_params + len(out_names)
    sharded = jax.jit(
        shard_map(
            _body,
            mesh=mesh,
            in_specs=(PartitionSpec("core"),) * nio,
            out_specs=(PartitionSpec("core"),) * len(out_names),
            check_rep=False,
        ),
        donate_argnums=donate,
        keep_unused=True,
    )
    zero_shapes = [
        ((n_cores * a.shape[0], *a.shape[1:]), a.dtype) for a in out_avals
    ]
    return sharded, in_names, out_names, zero_shapes


def _get_rt():
    if "rt" in _cache:
        return _cache["rt"]
    import jax
    from jax.sharding import Mesh, PartitionSpec, NamedSharding

    devices = jax.devices()[:NCORES]
    assert len(devices) == NCORES
    mesh = Mesh(np.asarray(devices), ("core",))
    sharding = NamedSharding(mesh, PartitionSpec("core"))
    nc1 = build_phase1()
    nc2 = build_phase2()
    exec1 = _make_exec(nc1, NCORES, mesh)
    exec2 = _make_exec(nc2, NCORES, mesh)
    _cache["rt"] = (sharding, exec1, exec2)
    return _cache["rt"]


def kernel(pcd):
    import os
    import time
    import jax

    tlog = []
    t0 = time.time()
    timing = bool(os.environ.get("KTIME"))

    def mark(label, val=None):
        if timing:
            if val is not None:
                jax.block_until_ready(val)
            tlog.append((label, time.time() - t0))

    pcd = np.asarray(pcd)
    assert pcd.shape == (B, N, 2), pcd.shape
    sharding, (ex1, in1, out1, z1), (ex2, in2, out2, z2) = _get_rt()
    mark("rt")

    q = _quantize_pack(pcd)  # [B, 2N + N/2] uint8
    mark("quantize")
    devq = jax.device_put(q, sharding)  # the one 40MB upload, reused by both phases
    mark("upload", devq)

    assert in1 == ["qpk"] and out1 == ["candv", "ccount"]
    candv_dev, cc = ex1(devq, np.zeros(*z1[0]), np.zeros(*z1[1]))
    mark("phase1", candv_dev)

    # phase 2 consumes the candidate list directly from device DRAM; the
    # candidate count only gates the (rare) host fallback, so its fetch
    # overlaps the phase-2 dispatch
    assert in2 == ["qpk", "cands"] and out2 == ["kvals"]
    (kv,) = ex2(devq, candv_dev, np.zeros(*z2[0]))
    try:
        cc.copy_to_host_async()
        kv.copy_to_host_async()
    except Exception:
        pass
    mark("phase2", kv)
    cc_np = np.asarray(cc).reshape(B)
    kv_np = np.asarray(kv).reshape(B)
    overflow = cc_np > CAND_CAP
    mark("kv_fetch")
    if timing:
        print(
            "KTIME "
            + " ".join(f"{l}={dt - p:.3f}" for (l, dt), p in zip(tlog, [0.0] + [d for _, d in tlog[:-1]])),
            flush=True,
        )

    out = np.zeros((B, TOPK, 1), np.float32)
    iota = np.arange(TOPK)
    for b in range(B):
        if overflow[b]:
            out[b, :, 0] = _host_exact(pcd[b])
        else:
            out[b, :, 0] = (iota < kv_np[b]).astype(np.float32)
    return out


# revision 30
# speedup vs baseline: 1.0635x; 1.0635x over previous
"""Trainium2 Bass kernel for nn_DeepMapping2D (histogram_binning).

Reference semantics: per cloud, quantize points to integer mm bins
(q = round_half_even(1000*p)), histogram into a 1024x1024 grid (shifted by
per-cloud coordinate minima), threshold counts (count/N > 2e-4 <=> count>=53),
sort the 0/1 occupancy descending, truncate to TOPK.  The sorted vector is K
ones then zeros, K = #bins with count >= 53.  Shifting by the minima is a
bijection on occupied bins, so K is shift-invariant and the device can work
on unshifted bin ids s = qx*1024 + qz (fine id, < 2^20).

Device algorithm (exact, two launches, all histogram work on device):

Phase 1 (screen): per cloud, the exact 2^14-bin coarse histogram H14 over
c14 = s>>6, computed as a PSUM-matmul scatter: per column of 128 points,
build 128-wide one-hots of hi7=c14>>7 and lo7=c14&127 by comparing a
constant iota row against the point's value (DVE tensor_scalar is_equal with
a per-partition scalar), then accumulate onehot_hi^T @ onehot_lo into PSUM
(bf16 0/1 inputs are exact; fp32 accumulation).  Only the thresholded int8
mask (H14 >= 53) goes back to the host.

Host: candidate cells = {c14 : H14[c14] >= 53} (every fine bin with count
>= 53 lives in one, since H14 upper-bounds its 64 fine bins).  ~1.4k/cloud
for the rbg-generated inputs.  Sorted, padded with -1 to NCHUNK*128 int16.

Phase 2 (refine): per cloud, exact fine counts for every candidate cell:
per column, one membership one-hot against the candidate row (int16
candidates streamed at DVE 4x, compared against the point's c14 as the
per-partition scalar) and one 64-wide one-hot of low6 = s&63; NCHUNK
matmuls accumulate membership^T @ onehot_low6 into PSUM -> exact
[candidate, low6] fine counts.  Threshold >= 53, count via ones^T @ mask
matmul, giving K per cloud.  The host formats the final rows (K ones then
zeros) from the device-computed K values.

Transport: the axon PJRT tunnel moves ~85 MB/s with ~75 ms per-call fixed
cost (a single sharded device_put beats any chunked/parallel scheme), so
the wall clock is dominated by host->device bytes.  The host pre-rounds
coordinates (bit-exact: f32 mult by 1000 + rint == the device's
+/-1.5*2^23 trick) and packs each point into 2.5 bytes: the low 8 bits of
qx and qz as an interleaved u8 plane, and the high 2 bits of each packed
as one nibble per point, two points per byte.  That is 40 MB instead of
the 128 MB raw f32 input; the device unpacks with exact f32 floor tricks
(a dozen cheap DVE ops per tile).  The upload happens ONCE: the packed q
lives on device as a committed sharded jax array reused by both phase
launches.  Both phases run through persistent jit(shard_map(bass_exec))
wrappers built once per process, so warm calls skip retrace/relower
entirely.

Host guards keep the kernel exact for arbitrary inputs: clouds whose
candidate count exceeds capacity fall back to an exact numpy recomputation
of that cloud.

Sharding: data-parallel over batch: 64 clouds -> 8 cores x 8 clouds.
"""

import numpy as np

B = 64
N = 262144
TOPK = 5120
NCORES = 8
CLOUDS_PER_CORE = B // NCORES
P = 128
GZ = 1024
NCHUNK = 12  # candidate capacity = NCHUNK*128 cells per cloud
CAND_CAP = NCHUNK * P
THRESH_COUNT = 53.0
C23 = 12582912.0  # 1.5 * 2^23

_cache = {}


def _chain(nc, tc, pools, qpk, c, F, mybir, need_low6, col0=0, n_points=N):
    """Elementwise chain for columns [col0, col0+F) of cloud c.

    qpk is the packed 2.5-byte/point format: per cloud, 2*n_points u8 of
    interleaved (qx&255, qz&255), then n_points/2 u8 of hi nibbles
    (qx>>8)*4+(qz>>8), two points per byte.  Computes s = qx*1024 + qz,
    c14 = s>>6 and optionally low6 = s&63, all exact in f32.
    """
    import concourse.bass as bass

    f32 = mybir.dt.float32
    u8 = mybir.dt.uint8
    op = mybir.AluOpType
    workp, chainp = pools
    FT = n_points // P  # full points-per-partition (column count)

    lo_src = qpk[c][: 2 * n_points].rearrange("(p x) -> p x", p=P)
    hp_src = qpk[c][2 * n_points :].rearrange("(p x) -> p x", p=P)
    tin = workp.tile([P, 2 * F], u8, tag="tin")
    nc.gpsimd.dma_start(out=tin[:], in_=lo_src[:, 2 * col0 : 2 * (col0 + F)])
    thp = workp.tile([P, F // 2], u8, tag="thp")
    nc.gpsimd.dma_start(out=thp[:], in_=hp_src[:, col0 // 2 : (col0 + F) // 2])

    # unpack hi nibbles: he = thp>>4, ho = thp&15 (floor via the mult +
    # offset + C23 round trick, all exact in f32), interleave to [P, F]
    t1 = chainp.tile([P, F // 2], f32, tag="t1")
    nc.vector.tensor_scalar(
        out=t1[:], in0=thp[:], scalar1=0.0625, scalar2=0.46875,
        op0=op.mult, op1=op.subtract,
    )
    he = chainp.tile([P, F // 2], f32, tag="he")
    nc.vector.tensor_scalar(
        out=he[:], in0=t1[:], scalar1=C23, scalar2=C23, op0=op.add, op1=op.subtract
    )
    ho = chainp.tile([P, F // 2], f32, tag="ho")
    nc.vector.scalar_tensor_tensor(
        out=ho[:], in0=he[:], scalar=-16.0, in1=thp[:], op0=op.mult, op1=op.add
    )
    hfull = chainp.tile([P, F], f32, tag="hfull")
    hv = hfull[:].rearrange("p (f two) -> p two f", two=2)
    nc.vector.tensor_copy(out=hv[:, 0], in_=he[:])
    nc.vector.tensor_copy(out=hv[:, 1], in_=ho[:])
    # hx = h>>2, hz = h&3
    t2 = chainp.tile([P, F], f32, tag="t2")
    nc.vector.tensor_scalar(
        out=t2[:], in0=hfull[:], scalar1=0.25, scalar2=0.375,
        op0=op.mult, op1=op.subtract,
    )
    hx = chainp.tile([P, F], f32, tag="hx")
    nc.vector.tensor_scalar(
        out=hx[:], in0=t2[:], scalar1=C23, scalar2=C23, op0=op.add, op1=op.subtract
    )
    hz = chainp.tile([P, F], f32, tag="hz")
    nc.vector.scalar_tensor_tensor(
        out=hz[:], in0=hx[:], scalar=-4.0, in1=hfull[:], op0=op.mult, op1=op.add
    )
    # qx = lox + 256*hx, qz = loz + 256*hz, s = qx*1024 + qz (exact, < 2^24)
    tv = tin[:].rearrange("p (f t) -> p t f", t=2)
    qx = chainp.tile([P, F], f32, tag="qx")
    nc.vector.scalar_tensor_tensor(
        out=qx[:], in0=hx[:], scalar=256.0, in1=tv[:, 0], op0=op.mult, op1=op.add
    )
    qz = chainp.tile([P, F], f32, tag="qz")
    nc.vector.scalar_tensor_tensor(
        out=qz[:], in0=hz[:], scalar=256.0, in1=tv[:, 1], op0=op.mult, op1=op.add
    )
    ts_ = chainp.tile([P, F], f32, tag="ts")
    nc.vector.scalar_tensor_tensor(
        out=ts_[:], in0=qx[:], scalar=1024.0, in1=qz[:], op0=op.mult, op1=op.add
    )
    # c14 = floor(s/64): s*2^-6 is exact, offset by -63/128 (exact), then the
    # fused (+C23, -C23) forces a round-to-nearest at integer granularity.
    tu = chainp.tile([P, F], f32, tag="tu")
    nc.vector.tensor_scalar(
        out=tu[:], in0=ts_[:], scalar1=0.015625, scalar2=0.4921875,
        op0=op.mult, op1=op.subtract,
    )
    tc14 = workp.tile([P, F], f32, tag="tc14")
    nc.vector.tensor_scalar(
        out=tc14[:], in0=tu[:], scalar1=C23, scalar2=C23, op0=op.add, op1=op.subtract
    )
    tlow6 = None
    if need_low6:
        # low6 = s - 64*c14
        tlow6 = workp.tile([P, F], f32, tag="tlow6")
        nc.vector.scalar_tensor_tensor(
            out=tlow6[:], in0=tc14[:], scalar=-64.0, in1=ts_[:],
            op0=op.mult, op1=op.add,
        )
    return tc14, tlow6, ts_


def build_phase1(n_clouds=CLOUDS_PER_CORE, n_points=N, nchunk=NCHUNK, unroll=32):
    """Per-cloud exact 2^14-bin coarse histogram -> threshold -> on-device
    compaction of candidate cell ids (sorted by cell id, -1 padded) plus the
    true candidate count.  candv stays in device DRAM for phase 2.

    Compaction: maskf[h,l] = [hist >= 53]; rc[h] = row count; prefix[h] =
    strict lower-triangular matmul over rc; off[h,l] = in-row exclusive
    prefix (log-shift scan); slot = prefix + off (position of cell (h,l) in
    the sorted candidate list).  For each l, a one-hot of slot against a
    0..CAP-1 iota row turns into two PSUM matmuls accumulating 128*h and l
    into candv[slot]; unused slots get -1 via the total count."""
    import concourse.bass as bass
    import concourse.mybir as mybir
    from concourse.tile import TileContext

    f32, bf16 = mybir.dt.float32, mybir.dt.bfloat16
    i16, i32 = mybir.dt.int16, mybir.dt.int32
    op = mybir.AluOpType
    F = n_points // P
    cap = nchunk * P

    from concourse import bacc

    nc = bacc.Bacc("TRN2", target_bir_lowering=False, debug=False)
    qpk = nc.declare_dram_parameter(
        "qpk", [n_clouds, 2 * n_points + n_points // 2], mybir.dt.uint8, isOutput=False
    )
    candv = nc.declare_dram_parameter("candv", [n_clouds, cap], i16, isOutput=True)
    ccount = nc.declare_dram_parameter("ccount", [1, n_clouds], f32, isOutput=True)
    pscr = nc.dram_tensor("pscr", [P], f32)

    with TileContext(nc) as tc:
        with (
            tc.tile_pool(name="const", bufs=1) as constp,
            tc.tile_pool(name="work", bufs=2) as workp,
            tc.tile_pool(name="chain", bufs=1) as chainp,
            tc.tile_pool(name="hilo", bufs=2) as hilop,
            tc.tile_pool(name="oh", bufs=8) as ohp,
            tc.tile_pool(name="cmp", bufs=2) as cmpp,
            tc.tile_pool(name="sel", bufs=4) as selp,
            tc.tile_pool(name="psum", bufs=2, space="PSUM") as psump,
            tc.tile_pool(name="cps", bufs=1, space="PSUM") as cpsp,
        ):
            iota_i = constp.tile([P, P], i32)
            nc.gpsimd.iota(iota_i[:], pattern=[[1, P]], base=0, channel_multiplier=0)
            iota_bf = constp.tile([P, P], bf16)
            nc.vector.tensor_copy(out=iota_bf[:], in_=iota_i[:])
            iota_f = constp.tile([P, P], f32)
            nc.vector.tensor_copy(out=iota_f[:], in_=iota_i[:])
            iotacap_i = constp.tile([P, cap], i32)
            nc.gpsimd.iota(iotacap_i[:], pattern=[[1, cap]], base=0, channel_multiplier=0)
            iotacap_f = constp.tile([P, cap], f32)
            nc.vector.tensor_copy(out=iotacap_f[:], in_=iotacap_i[:])
            ones_bf = constp.tile([P, 1], bf16)
            nc.vector.memset(ones_bf[:], 1.0)
            # per-partition index column via a DRAM roundtrip of an iota row
            nc.gpsimd.dma_start(out=pscr[:], in_=iota_f[0:1, :])
            piota_f = constp.tile([P, 1], f32)
            nc.gpsimd.dma_start(out=piota_f[:], in_=pscr[:].rearrange("(b o) -> b o", o=1))
            p128_bf = constp.tile([P, 1], bf16)
            nc.vector.tensor_scalar(
                out=p128_bf[:], in0=piota_f[:], scalar1=128.0, scalar2=None, op0=op.mult
            )
            # LT[k,h] = 1 if k < h (strict lower-triangular, as lhsT)
            lt_bf = constp.tile([P, P], bf16)
            nc.vector.tensor_scalar(
                out=lt_bf[:], in0=iota_bf[:], scalar1=piota_f[:, 0:1], scalar2=None,
                op0=op.is_gt,
            )
            cc_sb = constp.tile([1, n_clouds], f32)

            FC = min(512, F)  # chain chunk width (columns)
            for c in range(n_clouds):
                thi = hilop.tile([P, F], f32, tag="thi")
                tlo = hilop.tile([P, F], f32, tag="tlo")
                for col0 in range(0, F, FC):
                    tc14, _, _ = _chain(
                        nc, tc, (workp, chainp), qpk, c, FC, mybir,
                        need_low6=False, col0=col0, n_points=n_points,
                    )
                    # hi7 = floor(c14/128); lo7 = c14 - 128*hi7
                    thif = chainp.tile([P, FC], f32, tag="thif")
                    nc.vector.tensor_scalar(
                        out=thif[:], in0=tc14[:], scalar1=0.0078125,
                        scalar2=0.49609375, op0=op.mult, op1=op.subtract,
                    )
                    sl = slice(col0, col0 + FC)
                    nc.vector.tensor_scalar(
                        out=thi[:, sl], in0=thif[:], scalar1=C23, scalar2=C23,
                        op0=op.add, op1=op.subtract,
                    )
                    nc.vector.scalar_tensor_tensor(
                        out=tlo[:, sl], in0=thi[:, sl], scalar=-128.0,
                        in1=tc14[:], op0=op.mult, op1=op.add,
                    )
                hist = psump.tile([P, P], f32, tag="hist")
                nc.vector.memset(hist[:], 0.0)

                def body(iv, thi=thi, tlo=tlo, hist=hist):
                    ohh = ohp.tile([P, P], bf16, tag="ohh")
                    ohl = ohp.tile([P, P], bf16, tag="ohl")
                    nc.vector.tensor_scalar(
                        out=ohh[:], in0=iota_bf[:],
                        scalar1=thi[:, bass.ds(iv, 1)], scalar2=None,
                        op0=op.is_equal,
                    )
                    nc.vector.tensor_scalar(
                        out=ohl[:], in0=iota_bf[:],
                        scalar1=tlo[:, bass.ds(iv, 1)], scalar2=None,
                        op0=op.is_equal,
                    )
                    nc.tensor.matmul(
                        out=hist[:], lhsT=ohh[:], rhs=ohl[:],
                        start=False, stop=True, skip_group_check=True,
                    )

                tc.For_i_unrolled(0, F, 1, body, max_unroll=unroll)

                # ---- threshold + on-device candidate compaction ----
                maskf = cmpp.tile([P, P], f32, tag="maskf")
                nc.vector.tensor_scalar(
                    out=maskf[:], in0=hist[:], scalar1=THRESH_COUNT - 0.5,
                    scalar2=None, op0=op.is_ge,
                )
                rc = cmpp.tile([P, 1], f32, tag="rc")
                nc.vector.tensor_reduce(
                    out=rc[:], in_=maskf[:], axis=mybir.AxisListType.X, op=op.add
                )
                rc_bf = cmpp.tile([P, 1], bf16, tag="rcbf")
                nc.vector.tensor_copy(out=rc_bf[:], in_=rc[:])
                # prefix[h] = sum_{k<h} rc[k]; total = sum_k rc[k]
                pre_ps = cpsp.tile([P, 1], f32, tag="preps")
                nc.tensor.matmul(
                    out=pre_ps[:], lhsT=lt_bf[:], rhs=rc_bf[:], start=True, stop=True
                )
                tot_ps = cpsp.tile([1, 1], f32, tag="totps")
                nc.tensor.matmul(
                    out=tot_ps[:], lhsT=ones_bf[:], rhs=rc_bf[:], start=True, stop=True
                )
                pre_sb = cmpp.tile([P, 1], f32, tag="presb")
                nc.vector.tensor_copy(out=pre_sb[:], in_=pre_ps[:])
                tot_sb = cmpp.tile([1, 1], f32, tag="totsb")
                nc.vector.tensor_copy(out=tot_sb[:], in_=tot_ps[:])
                nc.vector.tensor_copy(out=cc_sb[0:1, c : c + 1], in_=tot_sb[:])
                # in-row inclusive scan (log shifts, ping-pong), then exclusive
                sA = cmpp.tile([P, P], f32, tag="scanA")
                sB = cmpp.tile([P, P], f32, tag="scanB")
                cur, nxt = sA, sB
                nc.vector.tensor_copy(out=cur[:], in_=maskf[:])
                for sh in (1, 2, 4, 8, 16, 32, 64):
                    nc.vector.tensor_copy(out=nxt[:, :sh], in_=cur[:, :sh])
                    nc.vector.tensor_tensor(
                        out=nxt[:, sh:], in0=cur[:, sh:], in1=cur[:, : P - sh],
                        op=op.add,
                    )
                    cur, nxt = nxt, cur
                # slot = prefix + (incl - mask); park non-candidates at 20000
                excl = cmpp.tile([P, P], f32, tag="excl")
                nc.vector.tensor_tensor(
                    out=excl[:], in0=cur[:], in1=maskf[:], op=op.subtract
                )
                slotA = cmpp.tile([P, P], f32, tag="slotA")
                nc.vector.tensor_scalar(
                    out=slotA[:], in0=excl[:], scalar1=pre_sb[:, 0:1],
                    scalar2=20000.0, op0=op.add, op1=op.subtract,
                )
                slotB = cmpp.tile([P, P], f32, tag="slotB")
                nc.vector.tensor_tensor(
                    out=slotB[:], in0=slotA[:], in1=maskf[:], op=op.mult
                )
                slot = cmpp.tile([P, P], f32, tag="slot")
                nc.vector.tensor_scalar(
                    out=slot[:], in0=slotB[:], scalar1=20000.0, scalar2=None, op0=op.add
                )
                # scatter cell ids to slots: candv[slot(h,l)] = 128*h + l
                cv_ps = cpsp.tile([1, cap], f32, tag="cvps")
                nc.vector.memset(cv_ps[:], 0.0)
                for l in range(P):
                    sel = selp.tile([P, cap], bf16, tag="sel")
                    nc.vector.tensor_scalar(
                        out=sel[:], in0=iotacap_f[:],
                        scalar1=slot[:, l : l + 1], scalar2=None, op0=op.is_equal,
                    )
                    for g in range(cap // 512):
                        gs = slice(g * 512, (g + 1) * 512)
                        nc.tensor.matmul(
                            out=cv_ps[:, gs], lhsT=p128_bf[:], rhs=sel[:, gs],
                            start=False, stop=True, skip_group_check=True,
                        )
                        nc.tensor.matmul(
                            out=cv_ps[:, gs], lhsT=iota_bf[:, l : l + 1], rhs=sel[:, gs],
                            start=False, stop=True, skip_group_check=True,
                        )
                # unused slots (j >= total) -> -1
                unused = cmpp.tile([1, cap], f32, tag="unused")
                nc.vector.tensor_scalar(
                    out=unused[:], in0=iotacap_f[0:1, :],
                    scalar1=tot_sb[0:1, 0:1], scalar2=None, op0=op.is_ge,
                )
                cfin = cmpp.tile([1, cap], f32, tag="cfin")
                nc.vector.tensor_tensor(
                    out=cfin[:], in0=cv_ps[:], in1=unused[:], op=op.subtract
                )
                cfin_i = cmpp.tile([1, cap], i16, tag="cfini")
                nc.vector.tensor_copy(out=cfin_i[:], in_=cfin[:])
                nc.gpsimd.dma_start(out=candv[c : c + 1, :], in_=cfin_i[:])

            nc.gpsimd.dma_start(out=ccount[:, :], in_=cc_sb[:])
    nc.compile()
    return nc


def build_phase2(n_clouds=CLOUDS_PER_CORE, n_points=N, nchunk=NCHUNK, unroll=16):
    """Exact [candidate,64] fine counts -> K per cloud."""
    import concourse.bass as bass
    import concourse.mybir as mybir
    from concourse.tile import TileContext

    f32, bf16 = mybir.dt.float32, mybir.dt.bfloat16
    i16, i32 = mybir.dt.int16, mybir.dt.int32
    op = mybir.AluOpType
    F = n_points // P
    cap = nchunk * P

    from concourse import bacc

    nc = bacc.Bacc("TRN2", target_bir_lowering=False, debug=False)
    qpk = nc.declare_dram_parameter(
        "qpk", [n_clouds, 2 * n_points + n_points // 2], mybir.dt.uint8, isOutput=False
    )
    cands = nc.declare_dram_parameter("cands", [n_clouds, cap], i16, isOutput=False)
    kvals = nc.declare_dram_parameter("kvals", [1, n_clouds], f32, isOutput=True)

    with TileContext(nc) as tc:
        with (
            tc.tile_pool(name="const", bufs=1) as constp,
            tc.tile_pool(name="work", bufs=2) as workp,
            tc.tile_pool(name="chain", bufs=1) as chainp,
            tc.tile_pool(name="oh", bufs=8) as ohp,
            tc.tile_pool(name="mk", bufs=4) as mkp,
            tc.tile_pool(name="psum", bufs=1, space="PSUM") as psump,
            tc.tile_pool(name="kps", bufs=1, space="PSUM") as kpsp,
        ):
            iota64_i = constp.tile([P, 64], i32)
            nc.gpsimd.iota(iota64_i[:], pattern=[[1, 64]], base=0, channel_multiplier=0)
            iota64_bf = constp.tile([P, 64], bf16)
            nc.vector.tensor_copy(out=iota64_bf[:], in_=iota64_i[:])
            ones_bf = constp.tile([P, 1], bf16)
            nc.vector.memset(ones_bf[:], 1.0)
            kv_sb = constp.tile([1, n_clouds], f32)

            for c in range(n_clouds):
                tc14, tlow6, _ = _chain(
                    nc, tc, (workp, chainp), qpk, c, F, mybir, need_low6=True,
                    n_points=n_points,
                )
                # candidate row broadcast to all partitions
                candbc = workp.tile([P, cap], i16, tag="candbc")
                cand_src = bass.AP(
                    tensor=cands.tensor if hasattr(cands, "tensor") else cands,
                    offset=c * cap,
                    ap=[[0, P], [1, cap]],
                )
                nc.gpsimd.dma_start(out=candbc[:], in_=cand_src)

                hist = psump.tile([P, cap], f32, tag="hist")
                nc.vector.memset(hist[:], 0.0)

                def body(iv):
                    memb = ohp.tile([P, cap], bf16, tag="memb")
                    loh = ohp.tile([P, 64], bf16, tag="loh")
                    nc.vector.tensor_scalar(
                        out=memb[:], in0=candbc[:],
                        scalar1=tc14[:, bass.ds(iv, 1)], scalar2=None,
                        op0=op.is_equal,
                    )
                    nc.vector.tensor_scalar(
                        out=loh[:], in0=iota64_bf[:],
                        scalar1=tlow6[:, bass.ds(iv, 1)], scalar2=None,
                        op0=op.is_equal,
                    )
                    # transposed accumulation: hist[w, cand] += loh^T @ memb,
                    # 512-wide moving slices so the 64-wide stationary loh is
                    # shared and PE streams at full width
                    for g in range(cap // 512):
                        nc.tensor.matmul(
                            out=hist[:64, g * 512 : (g + 1) * 512],
                            lhsT=loh[:],
                            rhs=memb[:, g * 512 : (g + 1) * 512],
                            start=False, stop=True, skip_group_check=True,
                        )

                tc.For_i_unrolled(0, F, 1, body, max_unroll=unroll)

                # K = sum over candidates/low6 of [count >= 53]
                kps = kpsp.tile([1, cap], f32, tag="kps")
                for g in range(cap // 512):
                    mask = mkp.tile([P, 512], bf16, tag="mask")
                    nc.vector.tensor_scalar(
                        out=mask[:64, :], in0=hist[:64, g * 512 : (g + 1) * 512],
                        scalar1=52.5, scalar2=None, op0=op.is_ge,
                    )
                    nc.tensor.matmul(
                        out=kps[:1, g * 512 : (g + 1) * 512],
                        lhsT=ones_bf[:64, :], rhs=mask[:64, :],
                        start=True, stop=True,
                    )
                nc.vector.tensor_reduce(
                    out=kv_sb[:1, c : c + 1], in_=kps[:],
                    axis=mybir.AxisListType.X, op=op.add,
                )

            nc.gpsimd.dma_start(out=kvals[:, :], in_=kv_sb[:])
    nc.compile()
    return nc


def build_fused(n_clouds=CLOUDS_PER_CORE, n_points=N, nchunk=NCHUNK, unroll=32):
    """Single-launch kernel: per cloud, coarse histogram -> threshold ->
    on-device candidate compaction -> fine refine -> K.  The candidate list
    never leaves the device (internal DRAM roundtrip broadcasts it across
    partitions); outputs are just kvals + ccount (a few bytes)."""
    import concourse.bass as bass
    import concourse.mybir as mybir
    from concourse.tile import TileContext

    f32, bf16 = mybir.dt.float32, mybir.dt.bfloat16
    i16, i32 = mybir.dt.int16, mybir.dt.int32
    op = mybir.AluOpType
    F = n_points // P
    cap = nchunk * P

    from concourse import bacc

    nc = bacc.Bacc("TRN2", target_bir_lowering=False, debug=False)
    qpk = nc.declare_dram_parameter(
        "qpk", [n_clouds, 2 * n_points + n_points // 2], mybir.dt.uint8, isOutput=False
    )
    kvals = nc.declare_dram_parameter("kvals", [1, n_clouds], f32, isOutput=True)
    ccount = nc.declare_dram_parameter("ccount", [1, n_clouds], f32, isOutput=True)
    candvs = nc.declare_dram_parameter("candvs", [n_clouds, cap], i16, isOutput=True)
    pscr = nc.dram_tensor("pscr", [P], f32)

    with TileContext(nc) as tc:
        with (
            tc.tile_pool(name="const", bufs=1) as constp,
            tc.tile_pool(name="work", bufs=2) as workp,
            tc.tile_pool(name="chain", bufs=1) as chainp,
            tc.tile_pool(name="hilo", bufs=2) as hilop,
            tc.tile_pool(name="oh", bufs=6) as ohp,
            tc.tile_pool(name="cmp", bufs=1) as cmpp,
            tc.tile_pool(name="row", bufs=1) as rowp,
            tc.tile_pool(name="sel", bufs=4) as selp,
            tc.tile_pool(name="mk", bufs=4) as mkp,
            tc.tile_pool(name="ps1", bufs=1, space="PSUM") as ps1p,
            tc.tile_pool(name="cps", bufs=1, space="PSUM") as cpsp,
            tc.tile_pool(name="ps2", bufs=1, space="PSUM") as ps2p,
        ):
            iota_i = constp.tile([P, P], i32)
            nc.gpsimd.iota(iota_i[:], pattern=[[1, P]], base=0, channel_multiplier=0)
            iota_bf = constp.tile([P, P], bf16)
            nc.vector.tensor_copy(out=iota_bf[:], in_=iota_i[:])
            iota_f = constp.tile([P, P], f32)
            nc.vector.tensor_copy(out=iota_f[:], in_=iota_i[:])
            iotacap_i = constp.tile([P, cap], i32)
            nc.gpsimd.iota(iotacap_i[:], pattern=[[1, cap]], base=0, channel_multiplier=0)
            iotacap_f = constp.tile([P, cap], f32)
            nc.vector.tensor_copy(out=iotacap_f[:], in_=iotacap_i[:])
            iota64_bf = constp.tile([P, 64], bf16)
            nc.vector.tensor_copy(out=iota64_bf[:], in_=iota_i[:, :64])
            ones_bf = constp.tile([P, 1], bf16)
            nc.vector.memset(ones_bf[:], 1.0)
            jge1 = constp.tile([1, cap], f32)
            nc.vector.tensor_scalar(
                out=jge1[:], in0=iotacap_f[0:1, :], scalar1=0.5, scalar2=None,
                op0=op.is_ge,
            )
            nc.gpsimd.dma_start(out=pscr[:], in_=iota_f[0:1, :])
            piota_f = constp.tile([P, 1], f32)
            nc.gpsimd.dma_start(out=piota_f[:], in_=pscr[:].rearrange("(b o) -> b o", o=1))
            p128_bf = constp.tile([P, 1], bf16)
            nc.vector.tensor_scalar(
                out=p128_bf[:], in0=piota_f[:], scalar1=128.0, scalar2=None, op0=op.mult
            )
            lt_bf = constp.tile([P, P], bf16)
            nc.vector.tensor_scalar(
                out=lt_bf[:], in0=iota_bf[:], scalar1=piota_f[:, 0:1], scalar2=None,
                op0=op.is_gt,
            )
            kv_sb = constp.tile([1, n_clouds], f32)
            cc_sb = constp.tile([1, n_clouds], f32)

            FC = min(512, F)
            for c in range(n_clouds):
                tc14 = workp.tile([P, F], f32, tag="tc14f")
                tlow6 = workp.tile([P, F], f32, tag="tlow6f")
                thi = hilop.tile([P, F], f32, tag="thi")
                tlo = hilop.tile([P, F], f32, tag="tlo")
                for col0 in range(0, F, FC):
                    sl = slice(col0, col0 + FC)
                    c14c, low6c, _ = _chain(
                        nc, tc, (workp, chainp), qpk, c, FC, mybir,
                        need_low6=True, col0=col0, n_points=n_points,
                    )
                    nc.vector.tensor_copy(out=tc14[:, sl], in_=c14c[:])
                    nc.vector.tensor_copy(out=tlow6[:, sl], in_=low6c[:])
                    thif = chainp.tile([P, FC], f32, tag="thif")
                    nc.vector.tensor_scalar(
                        out=thif[:], in0=c14c[:], scalar1=0.0078125,
                        scalar2=0.49609375, op0=op.mult, op1=op.subtract,
                    )
                    nc.vector.tensor_scalar(
                        out=thi[:, sl], in0=thif[:], scalar1=C23, scalar2=C23,
                        op0=op.add, op1=op.subtract,
                    )
                    nc.vector.scalar_tensor_tensor(
                        out=tlo[:, sl], in0=thi[:, sl], scalar=-128.0,
                        in1=c14c[:], op0=op.mult, op1=op.add,
                    )
                hist = ps1p.tile([P, P], f32, tag="hist")
                nc.vector.memset(hist[:], 0.0)

                def body1(iv, thi=thi, tlo=tlo, hist=hist):
                    ohh = ohp.tile([P, P], bf16, tag="ohh")
                    ohl = ohp.tile([P, P], bf16, tag="ohl")
                    nc.vector.tensor_scalar(
                        out=ohh[:], in0=iota_bf[:],
                        scalar1=thi[:, bass.ds(iv, 1)], scalar2=None, op0=op.is_equal,
                    )
                    nc.vector.tensor_scalar(
                        out=ohl[:], in0=iota_bf[:],
                        scalar1=tlo[:, bass.ds(iv, 1)], scalar2=None, op0=op.is_equal,
                    )
                    nc.tensor.matmul(
                        out=hist[:], lhsT=ohh[:], rhs=ohl[:],
                        start=False, stop=True, skip_group_check=True,
                    )

                tc.For_i_unrolled(0, F, 1, body1, max_unroll=unroll)

                # ---- threshold + compaction (see build_phase1) ----
                maskf = cmpp.tile([P, P], f32, tag="maskf")
                nc.vector.tensor_scalar(
                    out=maskf[:], in0=hist[:], scalar1=THRESH_COUNT - 0.5,
                    scalar2=None, op0=op.is_ge,
                )
                rc = cmpp.tile([P, 1], f32, tag="rc")
                nc.vector.tensor_reduce(
                    out=rc[:], in_=maskf[:], axis=mybir.AxisListType.X, op=op.add
                )
                rc_bf = cmpp.tile([P, 1], bf16, tag="rcbf")
                nc.vector.tensor_copy(out=rc_bf[:], in_=rc[:])
                pre_ps = cpsp.tile([P, 1], f32, tag="preps")
                nc.tensor.matmul(
                    out=pre_ps[:], lhsT=lt_bf[:], rhs=rc_bf[:], start=True, stop=True
                )
                pre_sb = cmpp.tile([P, 1], f32, tag="presb")
                nc.vector.tensor_copy(out=pre_sb[:], in_=pre_ps[:])
                sA = cmpp.tile([P, P], f32, tag="scanA")
                sB = cmpp.tile([P, P], f32, tag="scanB")
                cur, nxt = sA, sB
                nc.vector.tensor_copy(out=cur[:], in_=maskf[:])
                for sh in (1, 2, 4, 8, 16, 32, 64):
                    nc.vector.tensor_copy(out=nxt[:, :sh], in_=cur[:, :sh])
                    nc.vector.tensor_tensor(
                        out=nxt[:, sh:], in0=cur[:, sh:], in1=cur[:, : P - sh],
                        op=op.add,
                    )
                    cur, nxt = nxt, cur
                excl = cmpp.tile([P, P], f32, tag="excl")
                nc.vector.tensor_tensor(
                    out=excl[:], in0=cur[:], in1=maskf[:], op=op.subtract
                )
                slotA = cmpp.tile([P, P], f32, tag="slotA")
                nc.vector.tensor_scalar(
                    out=slotA[:], in0=excl[:], scalar1=pre_sb[:, 0:1],
                    scalar2=20000.0, op0=op.add, op1=op.subtract,
                )
                slotB = cmpp.tile([P, P], f32, tag="slotB")
                nc.vector.tensor_tensor(
                    out=slotB[:], in0=slotA[:], in1=maskf[:], op=op.mult
                )
                slot = cmpp.tile([P, P], f32, tag="slot")
                nc.vector.tensor_scalar(
                    out=slot[:], in0=slotB[:], scalar1=20000.0, scalar2=None, op0=op.add
                )
                cv_ps = cpsp.tile([1, cap], f32, tag="cvps")
                nc.vector.memset(cv_ps[:], 0.0)
                for l in range(P):
                    sel = selp.tile([P, cap], bf16, tag="sel")
                    nc.vector.tensor_scalar(
                        out=sel[:], in0=iotacap_f[:],
                        scalar1=slot[:, l : l + 1], scalar2=None, op0=op.is_equal,
                    )
                    for g in range(cap // 512):
                        gs = slice(g * 512, (g + 1) * 512)
                        nc.tensor.matmul(
                            out=cv_ps[:, gs], lhsT=p128_bf[:], rhs=sel[:, gs],
                            start=False, stop=True, skip_group_check=True,
                        )
                        nc.tensor.matmul(
                            out=cv_ps[:, gs], lhsT=iota_bf[:, l : l + 1], rhs=sel[:, gs],
                            start=False, stop=True, skip_group_check=True,
                        )
                # unused slots -> -1: cv==0 at j>=1 can only be an empty slot
                # (cell 0, the smallest id, always lands in slot 0 if present)
                zt = rowp.tile([1, cap], f32, tag="zt")
                nc.vector.tensor_scalar(
                    out=zt[:], in0=cv_ps[:], scalar1=0.5, scalar2=None, op0=op.is_lt
                )
                zz = rowp.tile([1, cap], f32, tag="zz")
                nc.vector.tensor_tensor(out=zz[:], in0=zt[:], in1=jge1[:], op=op.mult)
                cfin = rowp.tile([1, cap], f32, tag="cfin")
                nc.vector.tensor_tensor(
                    out=cfin[:], in0=cv_ps[:], in1=zz[:], op=op.subtract
                )
                cfin_i = rowp.tile([1, cap], i16, tag="cfini")
                nc.vector.tensor_copy(out=cfin_i[:], in_=cfin[:])
                cnz = rowp.tile([1, cap], f32, tag="cnz")
                nc.vector.tensor_scalar(
                    out=cnz[:], in0=cfin[:], scalar1=-0.5, scalar2=None, op0=op.is_ge
                )
                nc.vector.tensor_reduce(
                    out=cc_sb[0:1, c : c + 1], in_=cnz[:],
                    axis=mybir.AxisListType.X, op=op.add,
                )
                # broadcast the candidate row to all partitions via DRAM
                nc.gpsimd.dma_start(out=candvs[c : c + 1, :], in_=cfin_i[:])
                candbc = workp.tile([P, cap], i16, tag="candbc")
                cand_src = bass.AP(
                    tensor=candvs.tensor if hasattr(candvs, "tensor") else candvs,
                    offset=c * cap,
                    ap=[[0, P], [1, cap]],
                )
                nc.gpsimd.dma_start(out=candbc[:], in_=cand_src)

                # ---- fine refine (see build_phase2) ----
                hist2 = ps2p.tile([P, cap], f32, tag="hist2")
                nc.vector.memset(hist2[:], 0.0)

                def body2(iv, tc14=tc14, tlow6=tlow6, candbc=candbc, hist2=hist2):
                    memb = ohp.tile([P, cap], bf16, tag="memb")
                    loh = ohp.tile([P, 64], bf16, tag="loh")
                    nc.vector.tensor_scalar(
                        out=memb[:], in0=candbc[:],
                        scalar1=tc14[:, bass.ds(iv, 1)], scalar2=None, op0=op.is_equal,
                    )
                    nc.vector.tensor_scalar(
                        out=loh[:], in0=iota64_bf[:],
                        scalar1=tlow6[:, bass.ds(iv, 1)], scalar2=None, op0=op.is_equal,
                    )
                    for g in range(cap // 512):
                        nc.tensor.matmul(
                            out=hist2[:64, g * 512 : (g + 1) * 512],
                            lhsT=loh[:],
                            rhs=memb[:, g * 512 : (g + 1) * 512],
                            start=False, stop=True, skip_group_check=True,
                        )

                tc.For_i_unrolled(0, F, 1, body2, max_unroll=16)

                kps = cpsp.tile([1, cap], f32, tag="cvps")
                for g in range(cap // 512):
                    mask2 = mkp.tile([P, 512], bf16, tag="mask2")
                    nc.vector.tensor_scalar(
                        out=mask2[:64, :], in0=hist2[:64, g * 512 : (g + 1) * 512],
                        scalar1=52.5, scalar2=None, op0=op.is_ge,
                    )
                    nc.tensor.matmul(
                        out=kps[:1, g * 512 : (g + 1) * 512],
                        lhsT=ones_bf[:64, :], rhs=mask2[:64, :],
                        start=True, stop=True, skip_group_check=True,
                    )
                nc.vector.tensor_reduce(
                    out=kv_sb[:1, c : c + 1], in_=kps[:],
                    axis=mybir.AxisListType.X, op=op.add,
                )

            nc.gpsimd.dma_start(out=kvals[:, :], in_=kv_sb[:])
            nc.gpsimd.dma_start(out=ccount[:, :], in_=cc_sb[:])
    nc.compile()
    return nc


def _host_exact(points):
    """Exact numpy replica of the reference for one cloud. [N,2] f32 -> [TOPK]."""
    q = np.round(np.float32(1000.0) * points.astype(np.float32))
    xi = (q[:, 0] - q[:, 0].min()).astype(np.int64)
    zi = (q[:, 1] - q[:, 1].min()).astype(np.int64)
    idx = xi * GZ + zi
    counts = np.bincount(idx, minlength=1024 * GZ).astype(np.float32)
    occ = counts / np.float32(points.shape[0]) > np.float32(0.0002)
    k = min(int(occ.sum()), TOPK)
    out = np.zeros((TOPK,), np.float32)
    out[:k] = 1.0
    return out


def _numba_pack():
    if "nbpack" in _cache:
        return _cache["nbpack"]
    try:
        import numba

        @numba.njit(cache=False)
        def _nb_pack(pcd, out):
            nb, npts = pcd.shape[0], pcd.shape[1]
            for b in range(nb):
                base = 2 * npts
                for j in range(npts // 2):
                    n0 = 2 * j
                    qx0 = np.int32(np.rint(np.float32(1000.0) * pcd[b, n0, 0]))
                    qz0 = np.int32(np.rint(np.float32(1000.0) * pcd[b, n0, 1]))
                    qx1 = np.int32(np.rint(np.float32(1000.0) * pcd[b, n0 + 1, 0]))
                    qz1 = np.int32(np.rint(np.float32(1000.0) * pcd[b, n0 + 1, 1]))
                    out[b, 4 * j] = qx0 & 255
                    out[b, 4 * j + 1] = qz0 & 255
                    out[b, 4 * j + 2] = qx1 & 255
                    out[b, 4 * j + 3] = qz1 & 255
                    nib0 = ((qx0 >> 8) << 2) | (qz0 >> 8)
                    nib1 = ((qx1 >> 8) << 2) | (qz1 >> 8)
                    out[b, base + j] = (nib0 << 4) | nib1

        _cache["nbpack"] = _nb_pack
    except Exception:
        _cache["nbpack"] = None
    return _cache["nbpack"]


def _quantize_pack(pcd):
    """q = round_half_even(1000*pcd), bit-exact vs the reference's jnp.round
    (f32 multiply, then IEEE round-to-nearest-even), packed to 2.5 bytes per
    point: [2N u8 of interleaved low bytes | N/2 u8 of paired hi nibbles]."""
    nb = pcd.shape[0]
    npts = pcd.shape[1]
    out = np.empty((nb, 2 * npts + npts // 2), np.uint8)
    nbp = _numba_pack()
    if nbp is not None:
        nbp(pcd, out)
        return out
    t = np.empty((npts, 2), np.float32)
    for b in range(nb):
        np.multiply(pcd[b], np.float32(1000.0), out=t)
        np.rint(t, out=t)
        qi = t.astype(np.int16)  # [N, 2], values 0..1000
        v = qi.view(np.uint8)  # [N, 4]: qx_lo qx_hi qz_lo qz_hi
        lo = out[b, : 2 * npts].reshape(npts, 2)
        lo[:, 0] = v[:, 0]
        lo[:, 1] = v[:, 2]
        hb = (v[:, 1] << 2) | v[:, 3]
        out[b, 2 * npts :] = (hb[0::2] << 4) | hb[1::2]
    return out


def _make_exec(nc, n_cores, mesh):
    """Persistent jit(shard_map(bass_exec)) wrapper for a compiled Bass
    module: built once, reused every call (C++ fast-path dispatch after the
    first).  Mirrors concourse.bass2jax.run_bass_via_pjrt but accepts
    device-resident jax arrays so large inputs upload only once."""
    import jax
    import concourse.mybir as mybir
    from concourse import bass2jax
    from jax.sharding import PartitionSpec
    from jax.experimental.shard_map import shard_map

    bass2jax.install_neuronx_cc_hook()
    assert nc.dbg_addr is None and not nc.dbg_callbacks

    partition_name = nc.partition_id_tensor.name if nc.partition_id_tensor else None
    in_names, out_names, out_avals = [], [], []
    for alloc in nc.m.functions[0].allocations:
        if not isinstance(alloc, mybir.MemoryLocationSet):
            continue
        name = alloc.memorylocations[0].name
        if alloc.kind == "ExternalInput":
            if name != partition_name:
                in_names.append(name)
        elif alloc.kind == "ExternalOutput":
            out_names.append(name)
            out_avals.append(
                jax.core.ShapedArray(tuple(alloc.tensor_shape), mybir.dt.np(alloc.dtype))
            )
    n_params = len(in_names)
    all_names = in_names + out_names + ([partition_name] if partition_name else [])
    donate = tuple(range(n_params, n_params + len(out_names)))

    def _body(*args):
        operands = list(args)
        if partition_name is not None:
            operands.append(bass2jax.partition_id_tensor())
        return tuple(
            bass2jax._bass_exec_p.bind(
                *operands,
                out_avals=tuple(out_avals),
                in_names=tuple(all_names),
                out_names=tuple(out_names),
                lowering_input_output_aliases=(),
                sim_require_finite=True,
                sim_require_nnan=True,
                nc=nc,
            )
        )

    nio = n_params + len(out_names)
    sharded = jax.jit(
        shard_map(
            _body,
            mesh=mesh,
            in_specs=(PartitionSpec("core"),) * nio,
            out_specs=(PartitionSpec("core"),) * len(out_names),
            check_rep=False,
        ),
        donate_argnums=donate,
        keep_unused=True,
    )
    zero_shapes = [
        ((n_cores * a.shape[0], *a.shape[1:]), a.dtype) for a in out_avals
    ]
    return sharded, in_names, out_names, zero_shapes


def _get_rt():
    if "rt" in _cache:
        return _cache["rt"]
    import jax
    from jax.sharding import Mesh, PartitionSpec, NamedSharding

    devices = jax.devices()[:NCORES]
    assert len(devices) == NCORES
    mesh = Mesh(np.asarray(devices), ("core",))
    sharding = NamedSharding(mesh, PartitionSpec("core"))
    ncf = build_fused()
    execf = _make_exec(ncf, NCORES, mesh)
    _cache["rt"] = (sharding, execf)
    return _cache["rt"]


def kernel(pcd):
    import os
    import time
    import jax

    tlog = []
    t0 = time.time()
    timing = bool(os.environ.get("KTIME"))

    def mark(label, val=None):
        if timing:
            if val is not None:
                jax.block_until_ready(val)
            tlog.append((label, time.time() - t0))

    pcd = np.asarray(pcd)
    assert pcd.shape == (B, N, 2), pcd.shape
    sharding, (exf, inf, outf, zf) = _get_rt()
    mark("rt")

    q = _quantize_pack(pcd)  # [B, 2N + N/2] uint8
    mark("quantize")
    devq = jax.device_put(q, sharding)  # the one 40MB upload
    mark("upload", devq)

    assert inf == ["qpk"] and outf == ["kvals", "ccount", "candvs"]
    kv, cc, _candvs = exf(devq, np.zeros(*zf[0]), np.zeros(*zf[1]), np.zeros(*zf[2]))
    try:
        kv.copy_to_host_async()
        cc.copy_to_host_async()
    except Exception:
        pass
    mark("fused", kv)
    kv_np = np.asarray(kv).reshape(B)
    cc_np = np.asarray(cc).reshape(B)
    overflow = cc_np >= CAND_CAP
    mark("kv_fetch")
    if timing:
        print(
            "KTIME "
            + " ".join(f"{l}={dt - p:.3f}" for (l, dt), p in zip(tlog, [0.0] + [d for _, d in tlog[:-1]])),
            flush=True,
        )

    out = np.zeros((B, TOPK, 1), np.float32)
    iota = np.arange(TOPK)
    for b in range(B):
        if overflow[b]:
            out[b, :, 0] = _host_exact(pcd[b])
        else:
            out[b, :, 0] = (iota < kv_np[b]).astype(np.float32)
    return out


# revision 33
# speedup vs baseline: 1.1562x; 1.0871x over previous
"""Trainium2 Bass kernel for nn_DeepMapping2D (histogram_binning).

Reference semantics: per cloud, quantize points to integer mm bins
(q = round_half_even(1000*p)), histogram into a 1024x1024 grid (shifted by
per-cloud coordinate minima), threshold counts (count/N > 2e-4 <=> count>=53),
sort the 0/1 occupancy descending, truncate to TOPK.  The sorted vector is K
ones then zeros, K = #bins with count >= 53.  Shifting by the minima is a
bijection on occupied bins, so K is shift-invariant and the device can work
on unshifted bin ids s = qx*1024 + qz (fine id, < 2^20).

Device algorithm (exact, ONE launch per call, everything on device):

Screen: per cloud, the exact 2^14-bin coarse histogram H14 over c14 = s>>6,
computed as a PSUM-matmul scatter: per column of 128 points, build 128-wide
one-hots of hi7=c14>>7 and lo7=c14&127 by comparing a constant iota row
against the point's value (DVE tensor_scalar is_equal with a per-partition
scalar), then accumulate onehot_hi^T @ onehot_lo into PSUM (bf16 0/1 inputs
are exact; fp32 accumulation).

Compact (on device): candidate cells = {c14 : H14[c14] >= 53} (every fine
bin with count >= 53 lives in one, since H14 upper-bounds its 64 fine
bins; ~1.4k/cloud for the rbg inputs).  maskf -> per-row counts -> strict
lower-triangular matmul for the cross-partition prefix -> in-row log-shift
scan -> slot index per candidate cell; then for each lo7 column a one-hot
of slot against a 0..CAP-1 iota row feeds two PSUM matmuls that scatter
128*hi7 + lo7 into the compacted list; empty slots become -1 (a zero entry
at j>=1 can only be empty: cell 0, the smallest id, always sits at slot 0).
The list is broadcast to all partitions via a DRAM roundtrip.

Refine: per cloud, exact fine counts for every candidate cell: per column,
one membership one-hot against the candidate row (int16 candidates
streamed at DVE 4x, compared against the point's c14 as the per-partition
scalar) and one 64-wide one-hot of low6 = s&63; NCHUNK matmuls accumulate
membership^T @ onehot_low6 into PSUM -> exact [candidate, low6] fine
counts.  Threshold >= 53, count via ones^T @ mask matmul, giving K per
cloud.  Only kvals + ccount (a few bytes) return to the host, which
formats the final rows (K ones then zeros).

Transport: the axon PJRT tunnel moves ~85 MB/s with ~75 ms per-call fixed
cost (a single sharded device_put beats any chunked/parallel scheme), so
the wall clock is dominated by host->device bytes.  The host pre-rounds
coordinates (bit-exact: f32 mult by 1000 + rint == the device's
+/-1.5*2^23 trick) and packs each point into 2.5 bytes: the low 8 bits of
qx and qz as an interleaved u8 plane, and the high 2 bits of each packed
as one nibble per point, two points per byte.  That is 40 MB instead of
the 128 MB raw f32 input; the device unpacks with exact f32 floor tricks
(a dozen cheap DVE ops per tile).  The single fused launch runs through a
persistent jit(shard_map(bass_exec)) wrapper built once per process, so
warm calls skip retrace/relower entirely and touch the tunnel exactly
three times: one 40 MB upload, one launch, one tiny fetch.

Host guards keep the kernel exact for arbitrary inputs: clouds whose
candidate count exceeds capacity fall back to an exact numpy recomputation
of that cloud.

Sharding: data-parallel over batch: 64 clouds -> 8 cores x 8 clouds.
"""

import numpy as np

B = 64
N = 262144
TOPK = 5120
NCORES = 8
CLOUDS_PER_CORE = B // NCORES
P = 128
GZ = 1024
NCHUNK = 12  # candidate capacity = NCHUNK*128 cells per cloud
CAND_CAP = NCHUNK * P
THRESH_COUNT = 53.0
C23 = 12582912.0  # 1.5 * 2^23

_cache = {}


def _chain(nc, tc, pools, qpk, c, F, mybir, need_low6, col0=0, n_points=N):
    """Elementwise chain for columns [col0, col0+F) of cloud c.

    qpk is the packed 2.5-byte/point format: per cloud, 2*n_points u8 of
    interleaved (qx&255, qz&255), then n_points/2 u8 of hi nibbles
    (qx>>8)*4+(qz>>8), two points per byte.  Computes s = qx*1024 + qz,
    c14 = s>>6 and optionally low6 = s&63, all exact in f32.
    """
    import concourse.bass as bass

    f32 = mybir.dt.float32
    u8 = mybir.dt.uint8
    op = mybir.AluOpType
    workp, chainp = pools
    FT = n_points // P  # full points-per-partition (column count)

    lo_src = qpk[c][: 2 * n_points].rearrange("(p x) -> p x", p=P)
    hp_src = qpk[c][2 * n_points :].rearrange("(p x) -> p x", p=P)
    tin = workp.tile([P, 2 * F], u8, tag="tin")
    nc.gpsimd.dma_start(out=tin[:], in_=lo_src[:, 2 * col0 : 2 * (col0 + F)])
    thp = workp.tile([P, F // 2], u8, tag="thp")
    nc.gpsimd.dma_start(out=thp[:], in_=hp_src[:, col0 // 2 : (col0 + F) // 2])

    # unpack hi nibbles: he = thp>>4, ho = thp&15 (floor via the mult +
    # offset + C23 round trick, all exact in f32), interleave to [P, F]
    t1 = chainp.tile([P, F // 2], f32, tag="t1")
    nc.vector.tensor_scalar(
        out=t1[:], in0=thp[:], scalar1=0.0625, scalar2=0.46875,
        op0=op.mult, op1=op.subtract,
    )
    he = chainp.tile([P, F // 2], f32, tag="he")
    nc.vector.tensor_scalar(
        out=he[:], in0=t1[:], scalar1=C23, scalar2=C23, op0=op.add, op1=op.subtract
    )
    ho = chainp.tile([P, F // 2], f32, tag="ho")
    nc.vector.scalar_tensor_tensor(
        out=ho[:], in0=he[:], scalar=-16.0, in1=thp[:], op0=op.mult, op1=op.add
    )
    hfull = chainp.tile([P, F], f32, tag="hfull")
    hv = hfull[:].rearrange("p (f two) -> p two f", two=2)
    nc.vector.tensor_copy(out=hv[:, 0], in_=he[:])
    nc.vector.tensor_copy(out=hv[:, 1], in_=ho[:])
    # hx = h>>2, hz = h&3
    t2 = chainp.tile([P, F], f32, tag="t2")
    nc.vector.tensor_scalar(
        out=t2[:], in0=hfull[:], scalar1=0.25, scalar2=0.375,
        op0=op.mult, op1=op.subtract,
    )
    hx = chainp.tile([P, F], f32, tag="hx")
    nc.vector.tensor_scalar(
        out=hx[:], in0=t2[:], scalar1=C23, scalar2=C23, op0=op.add, op1=op.subtract
    )
    hz = chainp.tile([P, F], f32, tag="hz")
    nc.vector.scalar_tensor_tensor(
        out=hz[:], in0=hx[:], scalar=-4.0, in1=hfull[:], op0=op.mult, op1=op.add
    )
    # qx = lox + 256*hx, qz = loz + 256*hz, s = qx*1024 + qz (exact, < 2^24)
    tv = tin[:].rearrange("p (f t) -> p t f", t=2)
    qx = chainp.tile([P, F], f32, tag="qx")
    nc.vector.scalar_tensor_tensor(
        out=qx[:], in0=hx[:], scalar=256.0, in1=tv[:, 0], op0=op.mult, op1=op.add
    )
    qz = chainp.tile([P, F], f32, tag="qz")
    nc.vector.scalar_tensor_tensor(
        out=qz[:], in0=hz[:], scalar=256.0, in1=tv[:, 1], op0=op.mult, op1=op.add
    )
    ts_ = chainp.tile([P, F], f32, tag="ts")
    nc.vector.scalar_tensor_tensor(
        out=ts_[:], in0=qx[:], scalar=1024.0, in1=qz[:], op0=op.mult, op1=op.add
    )
    # c14 = floor(s/64): s*2^-6 is exact, offset by -63/128 (exact), then the
    # fused (+C23, -C23) forces a round-to-nearest at integer granularity.
    tu = chainp.tile([P, F], f32, tag="tu")
    nc.vector.tensor_scalar(
        out=tu[:], in0=ts_[:], scalar1=0.015625, scalar2=0.4921875,
        op0=op.mult, op1=op.subtract,
    )
    tc14 = workp.tile([P, F], f32, tag="tc14")
    nc.vector.tensor_scalar(
        out=tc14[:], in0=tu[:], scalar1=C23, scalar2=C23, op0=op.add, op1=op.subtract
    )
    tlow6 = None
    if need_low6:
        # low6 = s - 64*c14
        tlow6 = workp.tile([P, F], f32, tag="tlow6")
        nc.vector.scalar_tensor_tensor(
            out=tlow6[:], in0=tc14[:], scalar=-64.0, in1=ts_[:],
            op0=op.mult, op1=op.add,
        )
    return tc14, tlow6, ts_


def build_phase1(n_clouds=CLOUDS_PER_CORE, n_points=N, nchunk=NCHUNK, unroll=32):
    """Per-cloud exact 2^14-bin coarse histogram -> threshold -> on-device
    compaction of candidate cell ids (sorted by cell id, -1 padded) plus the
    true candidate count.  candv stays in device DRAM for phase 2.

    Compaction: maskf[h,l] = [hist >= 53]; rc[h] = row count; prefix[h] =
    strict lower-triangular matmul over rc; off[h,l] = in-row exclusive
    prefix (log-shift scan); slot = prefix + off (position of cell (h,l) in
    the sorted candidate list).  For each l, a one-hot of slot against a
    0..CAP-1 iota row turns into two PSUM matmuls accumulating 128*h and l
    into candv[slot]; unused slots get -1 via the total count."""
    import concourse.bass as bass
    import concourse.mybir as mybir
    from concourse.tile import TileContext

    f32, bf16 = mybir.dt.float32, mybir.dt.bfloat16
    i16, i32 = mybir.dt.int16, mybir.dt.int32
    op = mybir.AluOpType
    F = n_points // P
    cap = nchunk * P

    from concourse import bacc

    nc = bacc.Bacc("TRN2", target_bir_lowering=False, debug=False)
    qpk = nc.declare_dram_parameter(
        "qpk", [n_clouds, 2 * n_points + n_points // 2], mybir.dt.uint8, isOutput=False
    )
    candv = nc.declare_dram_parameter("candv", [n_clouds, cap], i16, isOutput=True)
    ccount = nc.declare_dram_parameter("ccount", [1, n_clouds], f32, isOutput=True)
    pscr = nc.dram_tensor("pscr", [P], f32)

    with TileContext(nc) as tc:
        with (
            tc.tile_pool(name="const", bufs=1) as constp,
            tc.tile_pool(name="work", bufs=2) as workp,
            tc.tile_pool(name="chain", bufs=1) as chainp,
            tc.tile_pool(name="hilo", bufs=2) as hilop,
            tc.tile_pool(name="oh", bufs=8) as ohp,
            tc.tile_pool(name="cmp", bufs=2) as cmpp,
            tc.tile_pool(name="sel", bufs=4) as selp,
            tc.tile_pool(name="psum", bufs=2, space="PSUM") as psump,
            tc.tile_pool(name="cps", bufs=1, space="PSUM") as cpsp,
        ):
            iota_i = constp.tile([P, P], i32)
            nc.gpsimd.iota(iota_i[:], pattern=[[1, P]], base=0, channel_multiplier=0)
            iota_bf = constp.tile([P, P], bf16)
            nc.vector.tensor_copy(out=iota_bf[:], in_=iota_i[:])
            iota_f = constp.tile([P, P], f32)
            nc.vector.tensor_copy(out=iota_f[:], in_=iota_i[:])
            iotacap_i = constp.tile([P, cap], i32)
            nc.gpsimd.iota(iotacap_i[:], pattern=[[1, cap]], base=0, channel_multiplier=0)
            iotacap_f = constp.tile([P, cap], f32)
            nc.vector.tensor_copy(out=iotacap_f[:], in_=iotacap_i[:])
            ones_bf = constp.tile([P, 1], bf16)
            nc.vector.memset(ones_bf[:], 1.0)
            # per-partition index column via a DRAM roundtrip of an iota row
            nc.gpsimd.dma_start(out=pscr[:], in_=iota_f[0:1, :])
            piota_f = constp.tile([P, 1], f32)
            nc.gpsimd.dma_start(out=piota_f[:], in_=pscr[:].rearrange("(b o) -> b o", o=1))
            p128_bf = constp.tile([P, 1], bf16)
            nc.vector.tensor_scalar(
                out=p128_bf[:], in0=piota_f[:], scalar1=128.0, scalar2=None, op0=op.mult
            )
            # LT[k,h] = 1 if k < h (strict lower-triangular, as lhsT)
            lt_bf = constp.tile([P, P], bf16)
            nc.vector.tensor_scalar(
                out=lt_bf[:], in0=iota_bf[:], scalar1=piota_f[:, 0:1], scalar2=None,
                op0=op.is_gt,
            )
            cc_sb = constp.tile([1, n_clouds], f32)

            FC = min(512, F)  # chain chunk width (columns)
            for c in range(n_clouds):
                thi = hilop.tile([P, F], f32, tag="thi")
                tlo = hilop.tile([P, F], f32, tag="tlo")
                for col0 in range(0, F, FC):
                    tc14, _, _ = _chain(
                        nc, tc, (workp, chainp), qpk, c, FC, mybir,
                        need_low6=False, col0=col0, n_points=n_points,
                    )
                    # hi7 = floor(c14/128); lo7 = c14 - 128*hi7
                    thif = chainp.tile([P, FC], f32, tag="thif")
                    nc.vector.tensor_scalar(
                        out=thif[:], in0=tc14[:], scalar1=0.0078125,
                        scalar2=0.49609375, op0=op.mult, op1=op.subtract,
                    )
                    sl = slice(col0, col0 + FC)
                    nc.vector.tensor_scalar(
                        out=thi[:, sl], in0=thif[:], scalar1=C23, scalar2=C23,
                        op0=op.add, op1=op.subtract,
                    )
                    nc.vector.scalar_tensor_tensor(
                        out=tlo[:, sl], in0=thi[:, sl], scalar=-128.0,
                        in1=tc14[:], op0=op.mult, op1=op.add,
                    )
                hist = psump.tile([P, P], f32, tag="hist")
                nc.vector.memset(hist[:], 0.0)

                def body(iv, thi=thi, tlo=tlo, hist=hist):
                    ohh = ohp.tile([P, P], bf16, tag="ohh")
                    ohl = ohp.tile([P, P], bf16, tag="ohl")
                    nc.vector.tensor_scalar(
                        out=ohh[:], in0=iota_bf[:],
                        scalar1=thi[:, bass.ds(iv, 1)], scalar2=None,
                        op0=op.is_equal,
                    )
                    nc.vector.tensor_scalar(
                        out=ohl[:], in0=iota_bf[:],
                        scalar1=tlo[:, bass.ds(iv, 1)], scalar2=None,
                        op0=op.is_equal,
                    )
                    nc.tensor.matmul(
                        out=hist[:], lhsT=ohh[:], rhs=ohl[:],
                        start=False, stop=True, skip_group_check=True,
                    )

                tc.For_i_unrolled(0, F, 1, body, max_unroll=unroll)

                # ---- threshold + on-device candidate compaction ----
                maskf = cmpp.tile([P, P], f32, tag="maskf")
                nc.vector.tensor_scalar(
                    out=maskf[:], in0=hist[:], scalar1=THRESH_COUNT - 0.5,
                    scalar2=None, op0=op.is_ge,
                )
                rc = cmpp.tile([P, 1], f32, tag="rc")
                nc.vector.tensor_reduce(
                    out=rc[:], in_=maskf[:], axis=mybir.AxisListType.X, op=op.add
                )
                rc_bf = cmpp.tile([P, 1], bf16, tag="rcbf")
                nc.vector.tensor_copy(out=rc_bf[:], in_=rc[:])
                # prefix[h] = sum_{k<h} rc[k]; total = sum_k rc[k]
                pre_ps = cpsp.tile([P, 1], f32, tag="preps")
                nc.tensor.matmul(
                    out=pre_ps[:], lhsT=lt_bf[:], rhs=rc_bf[:], start=True, stop=True
                )
                tot_ps = cpsp.tile([1, 1], f32, tag="totps")
                nc.tensor.matmul(
                    out=tot_ps[:], lhsT=ones_bf[:], rhs=rc_bf[:], start=True, stop=True
                )
                pre_sb = cmpp.tile([P, 1], f32, tag="presb")
                nc.vector.tensor_copy(out=pre_sb[:], in_=pre_ps[:])
                tot_sb = cmpp.tile([1, 1], f32, tag="totsb")
                nc.vector.tensor_copy(out=tot_sb[:], in_=tot_ps[:])
                nc.vector.tensor_copy(out=cc_sb[0:1, c : c + 1], in_=tot_sb[:])
                # in-row inclusive scan (log shifts, ping-pong), then exclusive
                sA = cmpp.tile([P, P], f32, tag="scanA")
                sB = cmpp.tile([P, P], f32, tag="scanB")
                cur, nxt = sA, sB
                nc.vector.tensor_copy(out=cur[:], in_=maskf[:])
                for sh in (1, 2, 4, 8, 16, 32, 64):
                    nc.vector.tensor_copy(out=nxt[:, :sh], in_=cur[:, :sh])
                    nc.vector.tensor_tensor(
                        out=nxt[:, sh:], in0=cur[:, sh:], in1=cur[:, : P - sh],
                        op=op.add,
                    )
                    cur, nxt = nxt, cur
                # slot = prefix + (incl - mask); park non-candidates at 20000
                excl = cmpp.tile([P, P], f32, tag="excl")
                nc.vector.tensor_tensor(
                    out=excl[:], in0=cur[:], in1=maskf[:], op=op.subtract
                )
                slotA = cmpp.tile([P, P], f32, tag="slotA")
                nc.vector.tensor_scalar(
                    out=slotA[:], in0=excl[:], scalar1=pre_sb[:, 0:1],
                    scalar2=20000.0, op0=op.add, op1=op.subtract,
                )
                slotB = cmpp.tile([P, P], f32, tag="slotB")
                nc.vector.tensor_tensor(
                    out=slotB[:], in0=slotA[:], in1=maskf[:], op=op.mult
                )
                slot = cmpp.tile([P, P], f32, tag="slot")
                nc.vector.tensor_scalar(
                    out=slot[:], in0=slotB[:], scalar1=20000.0, scalar2=None, op0=op.add
                )
                # scatter cell ids to slots: candv[slot(h,l)] = 128*h + l
                cv_ps = cpsp.tile([1, cap], f32, tag="cvps")
                nc.vector.memset(cv_ps[:], 0.0)
                for l in range(P):
                    sel = selp.tile([P, cap], bf16, tag="sel")
                    nc.vector.tensor_scalar(
                        out=sel[:], in0=iotacap_f[:],
                        scalar1=slot[:, l : l + 1], scalar2=None, op0=op.is_equal,
                    )
                    for g in range(cap // 512):
                        gs = slice(g * 512, (g + 1) * 512)
                        nc.tensor.matmul(
                            out=cv_ps[:, gs], lhsT=p128_bf[:], rhs=sel[:, gs],
                            start=False, stop=True, skip_group_check=True,
                        )
                        nc.tensor.matmul(
                            out=cv_ps[:, gs], lhsT=iota_bf[:, l : l + 1], rhs=sel[:, gs],
                            start=False, stop=True, skip_group_check=True,
                        )
                # unused slots (j >= total) -> -1
                unused = cmpp.tile([1, cap], f32, tag="unused")
                nc.vector.tensor_scalar(
                    out=unused[:], in0=iotacap_f[0:1, :],
                    scalar1=tot_sb[0:1, 0:1], scalar2=None, op0=op.is_ge,
                )
                cfin = cmpp.tile([1, cap], f32, tag="cfin")
                nc.vector.tensor_tensor(
                    out=cfin[:], in0=cv_ps[:], in1=unused[:], op=op.subtract
                )
                cfin_i = cmpp.tile([1, cap], i16, tag="cfini")
                nc.vector.tensor_copy(out=cfin_i[:], in_=cfin[:])
                nc.gpsimd.dma_start(out=candv[c : c + 1, :], in_=cfin_i[:])

            nc.gpsimd.dma_start(out=ccount[:, :], in_=cc_sb[:])
    nc.compile()
    return nc


def build_phase2(n_clouds=CLOUDS_PER_CORE, n_points=N, nchunk=NCHUNK, unroll=16):
    """Exact [candidate,64] fine counts -> K per cloud."""
    import concourse.bass as bass
    import concourse.mybir as mybir
    from concourse.tile import TileContext

    f32, bf16 = mybir.dt.float32, mybir.dt.bfloat16
    i16, i32 = mybir.dt.int16, mybir.dt.int32
    op = mybir.AluOpType
    F = n_points // P
    cap = nchunk * P

    from concourse import bacc

    nc = bacc.Bacc("TRN2", target_bir_lowering=False, debug=False)
    qpk = nc.declare_dram_parameter(
        "qpk", [n_clouds, 2 * n_points + n_points // 2], mybir.dt.uint8, isOutput=False
    )
    cands = nc.declare_dram_parameter("cands", [n_clouds, cap], i16, isOutput=False)
    kvals = nc.declare_dram_parameter("kvals", [1, n_clouds], f32, isOutput=True)

    with TileContext(nc) as tc:
        with (
            tc.tile_pool(name="const", bufs=1) as constp,
            tc.tile_pool(name="work", bufs=2) as workp,
            tc.tile_pool(name="chain", bufs=1) as chainp,
            tc.tile_pool(name="oh", bufs=8) as ohp,
            tc.tile_pool(name="mk", bufs=4) as mkp,
            tc.tile_pool(name="psum", bufs=1, space="PSUM") as psump,
            tc.tile_pool(name="kps", bufs=1, space="PSUM") as kpsp,
        ):
            iota64_i = constp.tile([P, 64], i32)
            nc.gpsimd.iota(iota64_i[:], pattern=[[1, 64]], base=0, channel_multiplier=0)
            iota64_bf = constp.tile([P, 64], bf16)
            nc.vector.tensor_copy(out=iota64_bf[:], in_=iota64_i[:])
            ones_bf = constp.tile([P, 1], bf16)
            nc.vector.memset(ones_bf[:], 1.0)
            kv_sb = constp.tile([1, n_clouds], f32)

            for c in range(n_clouds):
                tc14, tlow6, _ = _chain(
                    nc, tc, (workp, chainp), qpk, c, F, mybir, need_low6=True,
                    n_points=n_points,
                )
                # candidate row broadcast to all partitions
                candbc = workp.tile([P, cap], i16, tag="candbc")
                cand_src = bass.AP(
                    tensor=cands.tensor if hasattr(cands, "tensor") else cands,
                    offset=c * cap,
                    ap=[[0, P], [1, cap]],
                )
                nc.gpsimd.dma_start(out=candbc[:], in_=cand_src)

                hist = psump.tile([P, cap], f32, tag="hist")
                nc.vector.memset(hist[:], 0.0)

                def body(iv):
                    memb = ohp.tile([P, cap], bf16, tag="memb")
                    loh = ohp.tile([P, 64], bf16, tag="loh")
                    nc.vector.tensor_scalar(
                        out=memb[:], in0=candbc[:],
                        scalar1=tc14[:, bass.ds(iv, 1)], scalar2=None,
                        op0=op.is_equal,
                    )
                    nc.vector.tensor_scalar(
                        out=loh[:], in0=iota64_bf[:],
                        scalar1=tlow6[:, bass.ds(iv, 1)], scalar2=None,
                        op0=op.is_equal,
                    )
                    # transposed accumulation: hist[w, cand] += loh^T @ memb,
                    # 512-wide moving slices so the 64-wide stationary loh is
                    # shared and PE streams at full width
                    for g in range(cap // 512):
                        nc.tensor.matmul(
                            out=hist[:64, g * 512 : (g + 1) * 512],
                            lhsT=loh[:],
                            rhs=memb[:, g * 512 : (g + 1) * 512],
                            start=False, stop=True, skip_group_check=True,
                        )

                tc.For_i_unrolled(0, F, 1, body, max_unroll=unroll)

                # K = sum over candidates/low6 of [count >= 53]
                kps = kpsp.tile([1, cap], f32, tag="kps")
                for g in range(cap // 512):
                    mask = mkp.tile([P, 512], bf16, tag="mask")
                    nc.vector.tensor_scalar(
                        out=mask[:64, :], in0=hist[:64, g * 512 : (g + 1) * 512],
                        scalar1=52.5, scalar2=None, op0=op.is_ge,
                    )
                    nc.tensor.matmul(
                        out=kps[:1, g * 512 : (g + 1) * 512],
                        lhsT=ones_bf[:64, :], rhs=mask[:64, :],
                        start=True, stop=True,
                    )
                nc.vector.tensor_reduce(
                    out=kv_sb[:1, c : c + 1], in_=kps[:],
                    axis=mybir.AxisListType.X, op=op.add,
                )

            nc.gpsimd.dma_start(out=kvals[:, :], in_=kv_sb[:])
    nc.compile()
    return nc


def build_fused(n_clouds=CLOUDS_PER_CORE, n_points=N, nchunk=NCHUNK, unroll=32):
    """Single-launch kernel: per cloud, coarse histogram -> threshold ->
    on-device candidate compaction -> fine refine -> K.  The candidate list
    never leaves the device (internal DRAM roundtrip broadcasts it across
    partitions); outputs are just kvals + ccount (a few bytes)."""
    import concourse.bass as bass
    import concourse.mybir as mybir
    from concourse.tile import TileContext

    f32, bf16 = mybir.dt.float32, mybir.dt.bfloat16
    i16, i32 = mybir.dt.int16, mybir.dt.int32
    op = mybir.AluOpType
    F = n_points // P
    cap = nchunk * P

    from concourse import bacc

    nc = bacc.Bacc("TRN2", target_bir_lowering=False, debug=False)
    qpk = nc.declare_dram_parameter(
        "qpk", [n_clouds, 2 * n_points + n_points // 2], mybir.dt.uint8, isOutput=False
    )
    kvals = nc.declare_dram_parameter("kvals", [1, n_clouds], f32, isOutput=True)
    ccount = nc.declare_dram_parameter("ccount", [1, n_clouds], f32, isOutput=True)
    candvs = nc.declare_dram_parameter("candvs", [n_clouds, cap], i16, isOutput=True)
    pscr = nc.dram_tensor("pscr", [P], f32)

    with TileContext(nc) as tc:
        with (
            tc.tile_pool(name="const", bufs=1) as constp,
            tc.tile_pool(name="work", bufs=2) as workp,
            tc.tile_pool(name="chain", bufs=1) as chainp,
            tc.tile_pool(name="hilo", bufs=2) as hilop,
            tc.tile_pool(name="oh", bufs=6) as ohp,
            tc.tile_pool(name="cmp", bufs=1) as cmpp,
            tc.tile_pool(name="row", bufs=1) as rowp,
            tc.tile_pool(name="sel", bufs=4) as selp,
            tc.tile_pool(name="mk", bufs=4) as mkp,
            tc.tile_pool(name="ps1", bufs=1, space="PSUM") as ps1p,
            tc.tile_pool(name="cps", bufs=1, space="PSUM") as cpsp,
            tc.tile_pool(name="ps2", bufs=1, space="PSUM") as ps2p,
        ):
            iota_i = constp.tile([P, P], i32)
            nc.gpsimd.iota(iota_i[:], pattern=[[1, P]], base=0, channel_multiplier=0)
            iota_bf = constp.tile([P, P], bf16)
            nc.vector.tensor_copy(out=iota_bf[:], in_=iota_i[:])
            iota_f = constp.tile([P, P], f32)
            nc.vector.tensor_copy(out=iota_f[:], in_=iota_i[:])
            iotacap_i = constp.tile([P, cap], i32)
            nc.gpsimd.iota(iotacap_i[:], pattern=[[1, cap]], base=0, channel_multiplier=0)
            iotacap_f = constp.tile([P, cap], f32)
            nc.vector.tensor_copy(out=iotacap_f[:], in_=iotacap_i[:])
            iota64_bf = constp.tile([P, 64], bf16)
            nc.vector.tensor_copy(out=iota64_bf[:], in_=iota_i[:, :64])
            ones_bf = constp.tile([P, 1], bf16)
            nc.vector.memset(ones_bf[:], 1.0)
            jge1 = constp.tile([1, cap], f32)
            nc.vector.tensor_scalar(
                out=jge1[:], in0=iotacap_f[0:1, :], scalar1=0.5, scalar2=None,
                op0=op.is_ge,
            )
            nc.gpsimd.dma_start(out=pscr[:], in_=iota_f[0:1, :])
            piota_f = constp.tile([P, 1], f32)
            nc.gpsimd.dma_start(out=piota_f[:], in_=pscr[:].rearrange("(b o) -> b o", o=1))
            p128_bf = constp.tile([P, 1], bf16)
            nc.vector.tensor_scalar(
                out=p128_bf[:], in0=piota_f[:], scalar1=128.0, scalar2=None, op0=op.mult
            )
            lt_bf = constp.tile([P, P], bf16)
            nc.vector.tensor_scalar(
                out=lt_bf[:], in0=iota_bf[:], scalar1=piota_f[:, 0:1], scalar2=None,
                op0=op.is_gt,
            )
            kv_sb = constp.tile([1, n_clouds], f32)
            cc_sb = constp.tile([1, n_clouds], f32)

            FC = min(512, F)
            for c in range(n_clouds):
                tc14 = workp.tile([P, F], f32, tag="tc14f")
                tlow6 = workp.tile([P, F], f32, tag="tlow6f")
                thi = hilop.tile([P, F], f32, tag="thi")
                tlo = hilop.tile([P, F], f32, tag="tlo")
                for col0 in range(0, F, FC):
                    sl = slice(col0, col0 + FC)
                    c14c, low6c, _ = _chain(
                        nc, tc, (workp, chainp), qpk, c, FC, mybir,
                        need_low6=True, col0=col0, n_points=n_points,
                    )
                    nc.vector.tensor_copy(out=tc14[:, sl], in_=c14c[:])
                    nc.vector.tensor_copy(out=tlow6[:, sl], in_=low6c[:])
                    thif = chainp.tile([P, FC], f32, tag="thif")
                    nc.vector.tensor_scalar(
                        out=thif[:], in0=c14c[:], scalar1=0.0078125,
                        scalar2=0.49609375, op0=op.mult, op1=op.subtract,
                    )
                    nc.vector.tensor_scalar(
                        out=thi[:, sl], in0=thif[:], scalar1=C23, scalar2=C23,
                        op0=op.add, op1=op.subtract,
                    )
                    nc.vector.scalar_tensor_tensor(
                        out=tlo[:, sl], in0=thi[:, sl], scalar=-128.0,
                        in1=c14c[:], op0=op.mult, op1=op.add,
                    )
                hist = ps1p.tile([P, P], f32, tag="hist")
                nc.vector.memset(hist[:], 0.0)

                def body1(iv, thi=thi, tlo=tlo, hist=hist):
                    ohh = ohp.tile([P, P], bf16, tag="ohh")
                    ohl = ohp.tile([P, P], bf16, tag="ohl")
                    nc.vector.tensor_scalar(
                        out=ohh[:], in0=iota_bf[:],
                        scalar1=thi[:, bass.ds(iv, 1)], scalar2=None, op0=op.is_equal,
                    )
                    nc.vector.tensor_scalar(
                        out=ohl[:], in0=iota_bf[:],
                        scalar1=tlo[:, bass.ds(iv, 1)], scalar2=None, op0=op.is_equal,
                    )
                    nc.tensor.matmul(
                        out=hist[:], lhsT=ohh[:], rhs=ohl[:],
                        start=False, stop=True, skip_group_check=True,
                    )

                tc.For_i_unrolled(0, F, 1, body1, max_unroll=unroll)

                # ---- threshold + compaction (see build_phase1) ----
                maskf = cmpp.tile([P, P], f32, tag="maskf")
                nc.vector.tensor_scalar(
                    out=maskf[:], in0=hist[:], scalar1=THRESH_COUNT - 0.5,
                    scalar2=None, op0=op.is_ge,
                )
                rc = cmpp.tile([P, 1], f32, tag="rc")
                nc.vector.tensor_reduce(
                    out=rc[:], in_=maskf[:], axis=mybir.AxisListType.X, op=op.add
                )
                rc_bf = cmpp.tile([P, 1], bf16, tag="rcbf")
                nc.vector.tensor_copy(out=rc_bf[:], in_=rc[:])
                pre_ps = cpsp.tile([P, 1], f32, tag="preps")
                nc.tensor.matmul(
                    out=pre_ps[:], lhsT=lt_bf[:], rhs=rc_bf[:], start=True, stop=True
                )
                pre_sb = cmpp.tile([P, 1], f32, tag="presb")
                nc.vector.tensor_copy(out=pre_sb[:], in_=pre_ps[:])
                sA = cmpp.tile([P, P], f32, tag="scanA")
                sB = cmpp.tile([P, P], f32, tag="scanB")
                cur, nxt = sA, sB
                nc.vector.tensor_copy(out=cur[:], in_=maskf[:])
                for sh in (1, 2, 4, 8, 16, 32, 64):
                    nc.vector.tensor_copy(out=nxt[:, :sh], in_=cur[:, :sh])
                    nc.vector.tensor_tensor(
                        out=nxt[:, sh:], in0=cur[:, sh:], in1=cur[:, : P - sh],
                        op=op.add,
                    )
                    cur, nxt = nxt, cur
                excl = cmpp.tile([P, P], f32, tag="excl")
                nc.vector.tensor_tensor(
                    out=excl[:], in0=cur[:], in1=maskf[:], op=op.subtract
                )
                slotA = cmpp.tile([P, P], f32, tag="slotA")
                nc.vector.tensor_scalar(
                    out=slotA[:], in0=excl[:], scalar1=pre_sb[:, 0:1],
                    scalar2=20000.0, op0=op.add, op1=op.subtract,
                )
                slotB = cmpp.tile([P, P], f32, tag="slotB")
                nc.vector.tensor_tensor(
                    out=slotB[:], in0=slotA[:], in1=maskf[:], op=op.mult
                )
                slot = cmpp.tile([P, P], f32, tag="slot")
                nc.vector.tensor_scalar(
                    out=slot[:], in0=slotB[:], scalar1=20000.0, scalar2=None, op0=op.add
                )
                cv_ps = cpsp.tile([1, cap], f32, tag="cvps")
                nc.vector.memset(cv_ps[:], 0.0)
                for l in range(P):
                    sel = selp.tile([P, cap], bf16, tag="sel")
                    nc.vector.tensor_scalar(
                        out=sel[:], in0=iotacap_f[:],
                        scalar1=slot[:, l : l + 1], scalar2=None, op0=op.is_equal,
                    )
                    for g in range(cap // 512):
                        gs = slice(g * 512, (g + 1) * 512)
                        nc.tensor.matmul(
                            out=cv_ps[:, gs], lhsT=p128_bf[:], rhs=sel[:, gs],
                            start=False, stop=True, skip_group_check=True,
                        )
                        nc.tensor.matmul(
                            out=cv_ps[:, gs], lhsT=iota_bf[:, l : l + 1], rhs=sel[:, gs],
                            start=False, stop=True, skip_group_check=True,
                        )
                # unused slots -> -1: cv==0 at j>=1 can only be an empty slot
                # (cell 0, the smallest id, always lands in slot 0 if present)
                zt = rowp.tile([1, cap], f32, tag="zt")
                nc.vector.tensor_scalar(
                    out=zt[:], in0=cv_ps[:], scalar1=0.5, scalar2=None, op0=op.is_lt
                )
                zz = rowp.tile([1, cap], f32, tag="zz")
                nc.vector.tensor_tensor(out=zz[:], in0=zt[:], in1=jge1[:], op=op.mult)
                cfin = rowp.tile([1, cap], f32, tag="cfin")
                nc.vector.tensor_tensor(
                    out=cfin[:], in0=cv_ps[:], in1=zz[:], op=op.subtract
                )
                cfin_i = rowp.tile([1, cap], i16, tag="cfini")
                nc.vector.tensor_copy(out=cfin_i[:], in_=cfin[:])
                cnz = rowp.tile([1, cap], f32, tag="cnz")
                nc.vector.tensor_scalar(
                    out=cnz[:], in0=cfin[:], scalar1=-0.5, scalar2=None, op0=op.is_ge
                )
                nc.vector.tensor_reduce(
                    out=cc_sb[0:1, c : c + 1], in_=cnz[:],
                    axis=mybir.AxisListType.X, op=op.add,
                )
                # broadcast the candidate row to all partitions via DRAM
                nc.gpsimd.dma_start(out=candvs[c : c + 1, :], in_=cfin_i[:])
                candbc = workp.tile([P, cap], i16, tag="candbc")
                cand_src = bass.AP(
                    tensor=candvs.tensor if hasattr(candvs, "tensor") else candvs,
                    offset=c * cap,
                    ap=[[0, P], [1, cap]],
                )
                nc.gpsimd.dma_start(out=candbc[:], in_=cand_src)

                # ---- fine refine (see build_phase2) ----
                hist2 = ps2p.tile([P, cap], f32, tag="hist2")
                nc.vector.memset(hist2[:], 0.0)

                def body2(iv, tc14=tc14, tlow6=tlow6, candbc=candbc, hist2=hist2):
                    memb = ohp.tile([P, cap], bf16, tag="memb")
                    loh = ohp.tile([P, 64], bf16, tag="loh")
                    nc.vector.tensor_scalar(
                        out=memb[:], in0=candbc[:],
                        scalar1=tc14[:, bass.ds(iv, 1)], scalar2=None, op0=op.is_equal,
                    )
                    nc.vector.tensor_scalar(
                        out=loh[:], in0=iota64_bf[:],
                        scalar1=tlow6[:, bass.ds(iv, 1)], scalar2=None, op0=op.is_equal,
                    )
                    for g in range(cap // 512):
                        nc.tensor.matmul(
                            out=hist2[:64, g * 512 : (g + 1) * 512],
                            lhsT=loh[:],
                            rhs=memb[:, g * 512 : (g + 1) * 512],
                            start=False, stop=True, skip_group_check=True,
                        )

                tc.For_i_unrolled(0, F, 1, body2, max_unroll=16)

                kps = cpsp.tile([1, cap], f32, tag="cvps")
                for g in range(cap // 512):
                    mask2 = mkp.tile([P, 512], bf16, tag="mask2")
                    nc.vector.tensor_scalar(
                        out=mask2[:64, :], in0=hist2[:64, g * 512 : (g + 1) * 512],
                        scalar1=52.5, scalar2=None, op0=op.is_ge,
                    )
                    nc.tensor.matmul(
                        out=kps[:1, g * 512 : (g + 1) * 512],
                        lhsT=ones_bf[:64, :], rhs=mask2[:64, :],
                        start=True, stop=True, skip_group_check=True,
                    )
                nc.vector.tensor_reduce(
                    out=kv_sb[:1, c : c + 1], in_=kps[:],
                    axis=mybir.AxisListType.X, op=op.add,
                )

            nc.gpsimd.dma_start(out=kvals[:, :], in_=kv_sb[:])
            nc.gpsimd.dma_start(out=ccount[:, :], in_=cc_sb[:])
    nc.compile()
    return nc


def _host_exact(points):
    """Exact numpy replica of the reference for one cloud. [N,2] f32 -> [TOPK]."""
    q = np.round(np.float32(1000.0) * points.astype(np.float32))
    xi = (q[:, 0] - q[:, 0].min()).astype(np.int64)
    zi = (q[:, 1] - q[:, 1].min()).astype(np.int64)
    idx = xi * GZ + zi
    counts = np.bincount(idx, minlength=1024 * GZ).astype(np.float32)
    occ = counts / np.float32(points.shape[0]) > np.float32(0.0002)
    k = min(int(occ.sum()), TOPK)
    out = np.zeros((TOPK,), np.float32)
    out[:k] = 1.0
    return out


def _numba_pack():
    if "nbpack" in _cache:
        return _cache["nbpack"]
    try:
        import numba

        @numba.njit(cache=False)
        def _nb_pack(pcd, out):
            nb, npts = pcd.shape[0], pcd.shape[1]
            for b in range(nb):
                base = 2 * npts
                for j in range(npts // 2):
                    n0 = 2 * j
                    qx0 = np.int32(np.rint(np.float32(1000.0) * pcd[b, n0, 0]))
                    qz0 = np.int32(np.rint(np.float32(1000.0) * pcd[b, n0, 1]))
                    qx1 = np.int32(np.rint(np.float32(1000.0) * pcd[b, n0 + 1, 0]))
                    qz1 = np.int32(np.rint(np.float32(1000.0) * pcd[b, n0 + 1, 1]))
                    out[b, 4 * j] = qx0 & 255
                    out[b, 4 * j + 1] = qz0 & 255
                    out[b, 4 * j + 2] = qx1 & 255
                    out[b, 4 * j + 3] = qz1 & 255
                    nib0 = ((qx0 >> 8) << 2) | (qz0 >> 8)
                    nib1 = ((qx1 >> 8) << 2) | (qz1 >> 8)
                    out[b, base + j] = (nib0 << 4) | nib1

        _cache["nbpack"] = _nb_pack
    except Exception:
        _cache["nbpack"] = None
    return _cache["nbpack"]


def _quantize_pack(pcd):
    """q = round_half_even(1000*pcd), bit-exact vs the reference's jnp.round
    (f32 multiply, then IEEE round-to-nearest-even), packed to 2.5 bytes per
    point: [2N u8 of interleaved low bytes | N/2 u8 of paired hi nibbles]."""
    nb = pcd.shape[0]
    npts = pcd.shape[1]
    out = np.empty((nb, 2 * npts + npts // 2), np.uint8)
    nbp = _numba_pack()
    if nbp is not None:
        nbp(pcd, out)
        return out
    t = np.empty((npts, 2), np.float32)
    for b in range(nb):
        np.multiply(pcd[b], np.float32(1000.0), out=t)
        np.rint(t, out=t)
        qi = t.astype(np.int16)  # [N, 2], values 0..1000
        v = qi.view(np.uint8)  # [N, 4]: qx_lo qx_hi qz_lo qz_hi
        lo = out[b, : 2 * npts].reshape(npts, 2)
        lo[:, 0] = v[:, 0]
        lo[:, 1] = v[:, 2]
        hb = (v[:, 1] << 2) | v[:, 3]
        out[b, 2 * npts :] = (hb[0::2] << 4) | hb[1::2]
    return out


def _make_exec(nc, n_cores, mesh):
    """Persistent jit(shard_map(bass_exec)) wrapper for a compiled Bass
    module: built once, reused every call (C++ fast-path dispatch after the
    first).  Mirrors concourse.bass2jax.run_bass_via_pjrt but accepts
    device-resident jax arrays so large inputs upload only once."""
    import jax
    import concourse.mybir as mybir
    from concourse import bass2jax
    from jax.sharding import PartitionSpec
    from jax.experimental.shard_map import shard_map

    bass2jax.install_neuronx_cc_hook()
    assert nc.dbg_addr is None and not nc.dbg_callbacks

    partition_name = nc.partition_id_tensor.name if nc.partition_id_tensor else None
    in_names, out_names, out_avals = [], [], []
    for alloc in nc.m.functions[0].allocations:
        if not isinstance(alloc, mybir.MemoryLocationSet):
            continue
        name = alloc.memorylocations[0].name
        if alloc.kind == "ExternalInput":
            if name != partition_name:
                in_names.append(name)
        elif alloc.kind == "ExternalOutput":
            out_names.append(name)
            out_avals.append(
                jax.core.ShapedArray(tuple(alloc.tensor_shape), mybir.dt.np(alloc.dtype))
            )
    n_params = len(in_names)
    all_names = in_names + out_names + ([partition_name] if partition_name else [])
    donate = tuple(range(n_params, n_params + len(out_names)))

    def _body(*args):
        operands = list(args)
        if partition_name is not None:
            operands.append(bass2jax.partition_id_tensor())
        return tuple(
            bass2jax._bass_exec_p.bind(
                *operands,
                out_avals=tuple(out_avals),
                in_names=tuple(all_names),
                out_names=tuple(out_names),
                lowering_input_output_aliases=(),
                sim_require_finite=True,
                sim_require_nnan=True,
                nc=nc,
            )
        )

    nio = n_params + len(out_names)
    sharded = jax.jit(
        shard_map(
            _body,
            mesh=mesh,
            in_specs=(PartitionSpec("core"),) * nio,
            out_specs=(PartitionSpec("core"),) * len(out_names),
            check_rep=False,
        ),
        donate_argnums=donate,
        keep_unused=True,
    )
    zero_shapes = [
        ((n_cores * a.shape[0], *a.shape[1:]), a.dtype) for a in out_avals
    ]
    return sharded, in_names, out_names, zero_shapes


def _get_rt():
    if "rt" in _cache:
        return _cache["rt"]
    import jax
    from jax.sharding import Mesh, PartitionSpec, NamedSharding

    devices = jax.devices()[:NCORES]
    assert len(devices) == NCORES
    mesh = Mesh(np.asarray(devices), ("core",))
    sharding = NamedSharding(mesh, PartitionSpec("core"))
    ncf = build_fused()
    execf = _make_exec(ncf, NCORES, mesh)
    _cache["rt"] = (sharding, execf)
    return _cache["rt"]


def kernel(pcd):
    import os
    import time
    import jax

    tlog = []
    t0 = time.time()
    timing = bool(os.environ.get("KTIME"))

    def mark(label, val=None):
        if timing:
            if val is not None:
                jax.block_until_ready(val)
            tlog.append((label, time.time() - t0))

    pcd = np.ascontiguousarray(np.asarray(pcd), dtype=np.float32)
    assert pcd.shape == (B, N, 2), pcd.shape
    sharding, (exf, inf, outf, zf) = _get_rt()
    mark("rt")

    q = _quantize_pack(pcd)  # [B, 2N + N/2] uint8
    mark("quantize")
    devq = jax.device_put(q, sharding)  # the one 40MB upload
    mark("upload", devq)

    assert inf == ["qpk"] and outf == ["kvals", "ccount", "candvs"]
    kv, cc, _candvs = exf(devq, np.zeros(*zf[0]), np.zeros(*zf[1]), np.zeros(*zf[2]))
    try:
        kv.copy_to_host_async()
        cc.copy_to_host_async()
    except Exception:
        pass
    mark("fused", kv)
    kv_np = np.asarray(kv).reshape(B)
    cc_np = np.asarray(cc).reshape(B)
    overflow = cc_np >= CAND_CAP
    mark("kv_fetch")
    if timing:
        print(
            "KTIME "
            + " ".join(f"{l}={dt - p:.3f}" for (l, dt), p in zip(tlog, [0.0] + [d for _, d in tlog[:-1]])),
            flush=True,
        )

    out = np.zeros((B, TOPK, 1), np.float32)
    iota = np.arange(TOPK)
    for b in range(B):
        if overflow[b]:
            out[b, :, 0] = _host_exact(pcd[b])
        else:
            out[b, :, 0] = (iota < kv_np[b]).astype(np.float32)
    return out


# revision 34
# speedup vs baseline: 1.1621x; 1.0052x over previous
"""Trainium2 Bass kernel for nn_DeepMapping2D (histogram_binning).

Reference semantics: per cloud, quantize points to integer mm bins
(q = round_half_even(1000*p)), histogram into a 1024x1024 grid (shifted by
per-cloud coordinate minima), threshold counts (count/N > 2e-4 <=> count>=53),
sort the 0/1 occupancy descending, truncate to TOPK.  The sorted vector is K
ones then zeros, K = #bins with count >= 53.  Shifting by the minima is a
bijection on occupied bins, so K is shift-invariant and the device can work
on unshifted bin ids s = qx*1024 + qz (fine id, < 2^20).

Device algorithm (exact, ONE launch per call, everything on device):

Screen: per cloud, the exact 2^14-bin coarse histogram H14 over c14 = s>>6,
computed as a PSUM-matmul scatter: per column of 128 points, build 128-wide
one-hots of hi7=c14>>7 and lo7=c14&127 by comparing a constant iota row
against the point's value (DVE tensor_scalar is_equal with a per-partition
scalar), then accumulate onehot_hi^T @ onehot_lo into PSUM (bf16 0/1 inputs
are exact; fp32 accumulation).

Compact (on device): candidate cells = {c14 : H14[c14] >= 53} (every fine
bin with count >= 53 lives in one, since H14 upper-bounds its 64 fine
bins; ~1.4k/cloud for the rbg inputs).  maskf -> per-row counts -> strict
lower-triangular matmul for the cross-partition prefix -> in-row log-shift
scan -> slot index per candidate cell; then for each lo7 column a one-hot
of slot against a 0..CAP-1 iota row feeds two PSUM matmuls that scatter
128*hi7 + lo7 into the compacted list; empty slots become -1 (a zero entry
at j>=1 can only be empty: cell 0, the smallest id, always sits at slot 0).
The list is broadcast to all partitions via a DRAM roundtrip.

Refine: per cloud, exact fine counts for every candidate cell: per column,
one membership one-hot against the candidate row (int16 candidates
streamed at DVE 4x, compared against the point's c14 as the per-partition
scalar) and one 64-wide one-hot of low6 = s&63; NCHUNK matmuls accumulate
membership^T @ onehot_low6 into PSUM -> exact [candidate, low6] fine
counts.  Threshold >= 53, count via ones^T @ mask matmul, giving K per
cloud.  Only kvals + ccount (a few bytes) return to the host, which
formats the final rows (K ones then zeros).

Transport: the axon PJRT tunnel moves ~85 MB/s with ~75 ms per-call fixed
cost (a single sharded device_put beats any chunked/parallel scheme), so
the wall clock is dominated by host->device bytes.  The host pre-rounds
coordinates (bit-exact: f32 mult by 1000 + rint == the device's
+/-1.5*2^23 trick) and packs each point into 2.5 bytes: the low 8 bits of
qx and qz as an interleaved u8 plane, and the high 2 bits of each packed
as one nibble per point, two points per byte.  That is 40 MB instead of
the 128 MB raw f32 input; the device unpacks with exact f32 floor tricks
(a dozen cheap DVE ops per tile).  The single fused launch runs through a
persistent jit(shard_map(bass_exec)) wrapper built once per process, so
warm calls skip retrace/relower entirely and touch the tunnel exactly
three times: one 40 MB upload, one launch, one tiny fetch.

Host guards keep the kernel exact for arbitrary inputs: clouds whose
candidate count exceeds capacity fall back to an exact numpy recomputation
of that cloud.

Sharding: data-parallel over batch: 64 clouds -> 8 cores x 8 clouds.
"""

import numpy as np

B = 64
N = 262144
TOPK = 5120
NCORES = 8
CLOUDS_PER_CORE = B // NCORES
P = 128
GZ = 1024
NCHUNK = 12  # candidate capacity = NCHUNK*128 cells per cloud
CAND_CAP = NCHUNK * P
THRESH_COUNT = 53.0
C23 = 12582912.0  # 1.5 * 2^23

_cache = {}


def _chain(nc, tc, pools, qpk, c, F, mybir, need_low6, col0=0, n_points=N):
    """Elementwise chain for columns [col0, col0+F) of cloud c.

    qpk is the packed 2.5-byte/point format: per cloud, 2*n_points u8 of
    interleaved (qx&255, qz&255), then n_points/2 u8 of hi nibbles
    (qx>>8)*4+(qz>>8), two points per byte.  Computes s = qx*1024 + qz,
    c14 = s>>6 and optionally low6 = s&63, all exact in f32.
    """
    import concourse.bass as bass

    f32 = mybir.dt.float32
    u8 = mybir.dt.uint8
    op = mybir.AluOpType
    workp, chainp = pools
    FT = n_points // P  # full points-per-partition (column count)

    lo_src = qpk[c][: 2 * n_points].rearrange("(p x) -> p x", p=P)
    hp_src = qpk[c][2 * n_points :].rearrange("(p x) -> p x", p=P)
    tin = workp.tile([P, 2 * F], u8, tag="tin")
    nc.gpsimd.dma_start(out=tin[:], in_=lo_src[:, 2 * col0 : 2 * (col0 + F)])
    thp = workp.tile([P, F // 2], u8, tag="thp")
    nc.gpsimd.dma_start(out=thp[:], in_=hp_src[:, col0 // 2 : (col0 + F) // 2])

    # unpack hi nibbles: he = thp>>4, ho = thp&15 (floor via the mult +
    # offset + C23 round trick, all exact in f32), interleave to [P, F]
    t1 = chainp.tile([P, F // 2], f32, tag="t1")
    nc.vector.tensor_scalar(
        out=t1[:], in0=thp[:], scalar1=0.0625, scalar2=0.46875,
        op0=op.mult, op1=op.subtract,
    )
    he = chainp.tile([P, F // 2], f32, tag="he")
    nc.vector.tensor_scalar(
        out=he[:], in0=t1[:], scalar1=C23, scalar2=C23, op0=op.add, op1=op.subtract
    )
    ho = chainp.tile([P, F // 2], f32, tag="ho")
    nc.vector.scalar_tensor_tensor(
        out=ho[:], in0=he[:], scalar=-16.0, in1=thp[:], op0=op.mult, op1=op.add
    )
    hfull = chainp.tile([P, F], f32, tag="hfull")
    hv = hfull[:].rearrange("p (f two) -> p two f", two=2)
    nc.vector.tensor_copy(out=hv[:, 0], in_=he[:])
    nc.vector.tensor_copy(out=hv[:, 1], in_=ho[:])
    # hx = h>>2, hz = h&3
    t2 = chainp.tile([P, F], f32, tag="t2")
    nc.vector.tensor_scalar(
        out=t2[:], in0=hfull[:], scalar1=0.25, scalar2=0.375,
        op0=op.mult, op1=op.subtract,
    )
    hx = chainp.tile([P, F], f32, tag="hx")
    nc.vector.tensor_scalar(
        out=hx[:], in0=t2[:], scalar1=C23, scalar2=C23, op0=op.add, op1=op.subtract
    )
    hz = chainp.tile([P, F], f32, tag="hz")
    nc.vector.scalar_tensor_tensor(
        out=hz[:], in0=hx[:], scalar=-4.0, in1=hfull[:], op0=op.mult, op1=op.add
    )
    # qx = lox + 256*hx, qz = loz + 256*hz, s = qx*1024 + qz (exact, < 2^24)
    tv = tin[:].rearrange("p (f t) -> p t f", t=2)
    qx = chainp.tile([P, F], f32, tag="qx")
    nc.vector.scalar_tensor_tensor(
        out=qx[:], in0=hx[:], scalar=256.0, in1=tv[:, 0], op0=op.mult, op1=op.add
    )
    qz = chainp.tile([P, F], f32, tag="qz")
    nc.vector.scalar_tensor_tensor(
        out=qz[:], in0=hz[:], scalar=256.0, in1=tv[:, 1], op0=op.mult, op1=op.add
    )
    ts_ = chainp.tile([P, F], f32, tag="ts")
    nc.vector.scalar_tensor_tensor(
        out=ts_[:], in0=qx[:], scalar=1024.0, in1=qz[:], op0=op.mult, op1=op.add
    )
    # c14 = floor(s/64): s*2^-6 is exact, offset by -63/128 (exact), then the
    # fused (+C23, -C23) forces a round-to-nearest at integer granularity.
    tu = chainp.tile([P, F], f32, tag="tu")
    nc.vector.tensor_scalar(
        out=tu[:], in0=ts_[:], scalar1=0.015625, scalar2=0.4921875,
        op0=op.mult, op1=op.subtract,
    )
    tc14 = workp.tile([P, F], f32, tag="tc14")
    nc.vector.tensor_scalar(
        out=tc14[:], in0=tu[:], scalar1=C23, scalar2=C23, op0=op.add, op1=op.subtract
    )
    tlow6 = None
    if need_low6:
        # low6 = s - 64*c14
        tlow6 = workp.tile([P, F], f32, tag="tlow6")
        nc.vector.scalar_tensor_tensor(
            out=tlow6[:], in0=tc14[:], scalar=-64.0, in1=ts_[:],
            op0=op.mult, op1=op.add,
        )
    return tc14, tlow6, ts_


def build_phase1(n_clouds=CLOUDS_PER_CORE, n_points=N, nchunk=NCHUNK, unroll=32):
    """Per-cloud exact 2^14-bin coarse histogram -> threshold -> on-device
    compaction of candidate cell ids (sorted by cell id, -1 padded) plus the
    true candidate count.  candv stays in device DRAM for phase 2.

    Compaction: maskf[h,l] = [hist >= 53]; rc[h] = row count; prefix[h] =
    strict lower-triangular matmul over rc; off[h,l] = in-row exclusive
    prefix (log-shift scan); slot = prefix + off (position of cell (h,l) in
    the sorted candidate list).  For each l, a one-hot of slot against a
    0..CAP-1 iota row turns into two PSUM matmuls accumulating 128*h and l
    into candv[slot]; unused slots get -1 via the total count."""
    import concourse.bass as bass
    import concourse.mybir as mybir
    from concourse.tile import TileContext

    f32, bf16 = mybir.dt.float32, mybir.dt.bfloat16
    i16, i32 = mybir.dt.int16, mybir.dt.int32
    op = mybir.AluOpType
    F = n_points // P
    cap = nchunk * P

    from concourse import bacc

    nc = bacc.Bacc("TRN2", target_bir_lowering=False, debug=False)
    qpk = nc.declare_dram_parameter(
        "qpk", [n_clouds, 2 * n_points + n_points // 2], mybir.dt.uint8, isOutput=False
    )
    candv = nc.declare_dram_parameter("candv", [n_clouds, cap], i16, isOutput=True)
    ccount = nc.declare_dram_parameter("ccount", [1, n_clouds], f32, isOutput=True)
    pscr = nc.dram_tensor("pscr", [P], f32)

    with TileContext(nc) as tc:
        with (
            tc.tile_pool(name="const", bufs=1) as constp,
            tc.tile_pool(name="work", bufs=2) as workp,
            tc.tile_pool(name="chain", bufs=1) as chainp,
            tc.tile_pool(name="hilo", bufs=2) as hilop,
            tc.tile_pool(name="oh", bufs=8) as ohp,
            tc.tile_pool(name="cmp", bufs=2) as cmpp,
            tc.tile_pool(name="sel", bufs=4) as selp,
            tc.tile_pool(name="psum", bufs=2, space="PSUM") as psump,
            tc.tile_pool(name="cps", bufs=1, space="PSUM") as cpsp,
        ):
            iota_i = constp.tile([P, P], i32)
            nc.gpsimd.iota(iota_i[:], pattern=[[1, P]], base=0, channel_multiplier=0)
            iota_bf = constp.tile([P, P], bf16)
            nc.vector.tensor_copy(out=iota_bf[:], in_=iota_i[:])
            iota_f = constp.tile([P, P], f32)
            nc.vector.tensor_copy(out=iota_f[:], in_=iota_i[:])
            iotacap_i = constp.tile([P, cap], i32)
            nc.gpsimd.iota(iotacap_i[:], pattern=[[1, cap]], base=0, channel_multiplier=0)
            iotacap_f = constp.tile([P, cap], f32)
            nc.vector.tensor_copy(out=iotacap_f[:], in_=iotacap_i[:])
            ones_bf = constp.tile([P, 1], bf16)
            nc.vector.memset(ones_bf[:], 1.0)
            # per-partition index column via a DRAM roundtrip of an iota row
            nc.gpsimd.dma_start(out=pscr[:], in_=iota_f[0:1, :])
            piota_f = constp.tile([P, 1], f32)
            nc.gpsimd.dma_start(out=piota_f[:], in_=pscr[:].rearrange("(b o) -> b o", o=1))
            p128_bf = constp.tile([P, 1], bf16)
            nc.vector.tensor_scalar(
                out=p128_bf[:], in0=piota_f[:], scalar1=128.0, scalar2=None, op0=op.mult
            )
            # LT[k,h] = 1 if k < h (strict lower-triangular, as lhsT)
            lt_bf = constp.tile([P, P], bf16)
            nc.vector.tensor_scalar(
                out=lt_bf[:], in0=iota_bf[:], scalar1=piota_f[:, 0:1], scalar2=None,
                op0=op.is_gt,
            )
            cc_sb = constp.tile([1, n_clouds], f32)

            FC = min(512, F)  # chain chunk width (columns)
            for c in range(n_clouds):
                thi = hilop.tile([P, F], f32, tag="thi")
                tlo = hilop.tile([P, F], f32, tag="tlo")
                for col0 in range(0, F, FC):
                    tc14, _, _ = _chain(
                        nc, tc, (workp, chainp), qpk, c, FC, mybir,
                        need_low6=False, col0=col0, n_points=n_points,
                    )
                    # hi7 = floor(c14/128); lo7 = c14 - 128*hi7
                    thif = chainp.tile([P, FC], f32, tag="thif")
                    nc.vector.tensor_scalar(
                        out=thif[:], in0=tc14[:], scalar1=0.0078125,
                        scalar2=0.49609375, op0=op.mult, op1=op.subtract,
                    )
                    sl = slice(col0, col0 + FC)
                    nc.vector.tensor_scalar(
                        out=thi[:, sl], in0=thif[:], scalar1=C23, scalar2=C23,
                        op0=op.add, op1=op.subtract,
                    )
                    nc.vector.scalar_tensor_tensor(
                        out=tlo[:, sl], in0=thi[:, sl], scalar=-128.0,
                        in1=tc14[:], op0=op.mult, op1=op.add,
                    )
                hist = psump.tile([P, P], f32, tag="hist")
                nc.vector.memset(hist[:], 0.0)

                def body(iv, thi=thi, tlo=tlo, hist=hist):
                    ohh = ohp.tile([P, P], bf16, tag="ohh")
                    ohl = ohp.tile([P, P], bf16, tag="ohl")
                    nc.vector.tensor_scalar(
                        out=ohh[:], in0=iota_bf[:],
                        scalar1=thi[:, bass.ds(iv, 1)], scalar2=None,
                        op0=op.is_equal,
                    )
                    nc.vector.tensor_scalar(
                        out=ohl[:], in0=iota_bf[:],
                        scalar1=tlo[:, bass.ds(iv, 1)], scalar2=None,
                        op0=op.is_equal,
                    )
                    nc.tensor.matmul(
                        out=hist[:], lhsT=ohh[:], rhs=ohl[:],
                        start=False, stop=True, skip_group_check=True,
                    )

                tc.For_i_unrolled(0, F, 1, body, max_unroll=unroll)

                # ---- threshold + on-device candidate compaction ----
                maskf = cmpp.tile([P, P], f32, tag="maskf")
                nc.vector.tensor_scalar(
                    out=maskf[:], in0=hist[:], scalar1=THRESH_COUNT - 0.5,
                    scalar2=None, op0=op.is_ge,
                )
                rc = cmpp.tile([P, 1], f32, tag="rc")
                nc.vector.tensor_reduce(
                    out=rc[:], in_=maskf[:], axis=mybir.AxisListType.X, op=op.add
                )
                rc_bf = cmpp.tile([P, 1], bf16, tag="rcbf")
                nc.vector.tensor_copy(out=rc_bf[:], in_=rc[:])
                # prefix[h] = sum_{k<h} rc[k]; total = sum_k rc[k]
                pre_ps = cpsp.tile([P, 1], f32, tag="preps")
                nc.tensor.matmul(
                    out=pre_ps[:], lhsT=lt_bf[:], rhs=rc_bf[:], start=True, stop=True
                )
                tot_ps = cpsp.tile([1, 1], f32, tag="totps")
                nc.tensor.matmul(
                    out=tot_ps[:], lhsT=ones_bf[:], rhs=rc_bf[:], start=True, stop=True
                )
                pre_sb = cmpp.tile([P, 1], f32, tag="presb")
                nc.vector.tensor_copy(out=pre_sb[:], in_=pre_ps[:])
                tot_sb = cmpp.tile([1, 1], f32, tag="totsb")
                nc.vector.tensor_copy(out=tot_sb[:], in_=tot_ps[:])
                nc.vector.tensor_copy(out=cc_sb[0:1, c : c + 1], in_=tot_sb[:])
                # in-row inclusive scan (log shifts, ping-pong), then exclusive
                sA = cmpp.tile([P, P], f32, tag="scanA")
                sB = cmpp.tile([P, P], f32, tag="scanB")
                cur, nxt = sA, sB
                nc.vector.tensor_copy(out=cur[:], in_=maskf[:])
                for sh in (1, 2, 4, 8, 16, 32, 64):
                    nc.vector.tensor_copy(out=nxt[:, :sh], in_=cur[:, :sh])
                    nc.vector.tensor_tensor(
                        out=nxt[:, sh:], in0=cur[:, sh:], in1=cur[:, : P - sh],
                        op=op.add,
                    )
                    cur, nxt = nxt, cur
                # slot = prefix + (incl - mask); park non-candidates at 20000
                excl = cmpp.tile([P, P], f32, tag="excl")
                nc.vector.tensor_tensor(
                    out=excl[:], in0=cur[:], in1=maskf[:], op=op.subtract
                )
                slotA = cmpp.tile([P, P], f32, tag="slotA")
                nc.vector.tensor_scalar(
                    out=slotA[:], in0=excl[:], scalar1=pre_sb[:, 0:1],
                    scalar2=20000.0, op0=op.add, op1=op.subtract,
                )
                slotB = cmpp.tile([P, P], f32, tag="slotB")
                nc.vector.tensor_tensor(
                    out=slotB[:], in0=slotA[:], in1=maskf[:], op=op.mult
                )
                slot = cmpp.tile([P, P], f32, tag="slot")
                nc.vector.tensor_scalar(
                    out=slot[:], in0=slotB[:], scalar1=20000.0, scalar2=None, op0=op.add
                )
                # scatter cell ids to slots: candv[slot(h,l)] = 128*h + l
                cv_ps = cpsp.tile([1, cap], f32, tag="cvps")
                nc.vector.memset(cv_ps[:], 0.0)
                for l in range(P):
                    sel = selp.tile([P, cap], bf16, tag="sel")
                    nc.vector.tensor_scalar(
                        out=sel[:], in0=iotacap_f[:],
                        scalar1=slot[:, l : l + 1], scalar2=None, op0=op.is_equal,
                    )
                    for g in range(cap // 512):
                        gs = slice(g * 512, (g + 1) * 512)
                        nc.tensor.matmul(
                            out=cv_ps[:, gs], lhsT=p128_bf[:], rhs=sel[:, gs],
                            start=False, stop=True, skip_group_check=True,
                        )
                        nc.tensor.matmul(
                            out=cv_ps[:, gs], lhsT=iota_bf[:, l : l + 1], rhs=sel[:, gs],
                            start=False, stop=True, skip_group_check=True,
                        )
                # unused slots (j >= total) -> -1
                unused = cmpp.tile([1, cap], f32, tag="unused")
                nc.vector.tensor_scalar(
                    out=unused[:], in0=iotacap_f[0:1, :],
                    scalar1=tot_sb[0:1, 0:1], scalar2=None, op0=op.is_ge,
                )
                cfin = cmpp.tile([1, cap], f32, tag="cfin")
                nc.vector.tensor_tensor(
                    out=cfin[:], in0=cv_ps[:], in1=unused[:], op=op.subtract
                )
                cfin_i = cmpp.tile([1, cap], i16, tag="cfini")
                nc.vector.tensor_copy(out=cfin_i[:], in_=cfin[:])
                nc.gpsimd.dma_start(out=candv[c : c + 1, :], in_=cfin_i[:])

            nc.gpsimd.dma_start(out=ccount[:, :], in_=cc_sb[:])
    nc.compile()
    return nc


def build_phase2(n_clouds=CLOUDS_PER_CORE, n_points=N, nchunk=NCHUNK, unroll=16):
    """Exact [candidate,64] fine counts -> K per cloud."""
    import concourse.bass as bass
    import concourse.mybir as mybir
    from concourse.tile import TileContext

    f32, bf16 = mybir.dt.float32, mybir.dt.bfloat16
    i16, i32 = mybir.dt.int16, mybir.dt.int32
    op = mybir.AluOpType
    F = n_points // P
    cap = nchunk * P

    from concourse import bacc

    nc = bacc.Bacc("TRN2", target_bir_lowering=False, debug=False)
    qpk = nc.declare_dram_parameter(
        "qpk", [n_clouds, 2 * n_points + n_points // 2], mybir.dt.uint8, isOutput=False
    )
    cands = nc.declare_dram_parameter("cands", [n_clouds, cap], i16, isOutput=False)
    kvals = nc.declare_dram_parameter("kvals", [1, n_clouds], f32, isOutput=True)

    with TileContext(nc) as tc:
        with (
            tc.tile_pool(name="const", bufs=1) as constp,
            tc.tile_pool(name="work", bufs=2) as workp,
            tc.tile_pool(name="chain", bufs=1) as chainp,
            tc.tile_pool(name="oh", bufs=8) as ohp,
            tc.tile_pool(name="mk", bufs=4) as mkp,
            tc.tile_pool(name="psum", bufs=1, space="PSUM") as psump,
            tc.tile_pool(name="kps", bufs=1, space="PSUM") as kpsp,
        ):
            iota64_i = constp.tile([P, 64], i32)
            nc.gpsimd.iota(iota64_i[:], pattern=[[1, 64]], base=0, channel_multiplier=0)
            iota64_bf = constp.tile([P, 64], bf16)
            nc.vector.tensor_copy(out=iota64_bf[:], in_=iota64_i[:])
            ones_bf = constp.tile([P, 1], bf16)
            nc.vector.memset(ones_bf[:], 1.0)
            kv_sb = constp.tile([1, n_clouds], f32)

            for c in range(n_clouds):
                tc14, tlow6, _ = _chain(
                    nc, tc, (workp, chainp), qpk, c, F, mybir, need_low6=True,
                    n_points=n_points,
                )
                # candidate row broadcast to all partitions
                candbc = workp.tile([P, cap], i16, tag="candbc")
                cand_src = bass.AP(
                    tensor=cands.tensor if hasattr(cands, "tensor") else cands,
                    offset=c * cap,
                    ap=[[0, P], [1, cap]],
                )
                nc.gpsimd.dma_start(out=candbc[:], in_=cand_src)

                hist = psump.tile([P, cap], f32, tag="hist")
                nc.vector.memset(hist[:], 0.0)

                def body(iv):
                    memb = ohp.tile([P, cap], bf16, tag="memb")
                    loh = ohp.tile([P, 64], bf16, tag="loh")
                    nc.vector.tensor_scalar(
                        out=memb[:], in0=candbc[:],
                        scalar1=tc14[:, bass.ds(iv, 1)], scalar2=None,
                        op0=op.is_equal,
                    )
                    nc.vector.tensor_scalar(
                        out=loh[:], in0=iota64_bf[:],
                        scalar1=tlow6[:, bass.ds(iv, 1)], scalar2=None,
                        op0=op.is_equal,
                    )
                    # transposed accumulation: hist[w, cand] += loh^T @ memb,
                    # 512-wide moving slices so the 64-wide stationary loh is
                    # shared and PE streams at full width
                    for g in range(cap // 512):
                        nc.tensor.matmul(
                            out=hist[:64, g * 512 : (g + 1) * 512],
                            lhsT=loh[:],
                            rhs=memb[:, g * 512 : (g + 1) * 512],
                            start=False, stop=True, skip_group_check=True,
                        )

                tc.For_i_unrolled(0, F, 1, body, max_unroll=unroll)

                # K = sum over candidates/low6 of [count >= 53]
                kps = kpsp.tile([1, cap], f32, tag="kps")
                for g in range(cap // 512):
                    mask = mkp.tile([P, 512], bf16, tag="mask")
                    nc.vector.tensor_scalar(
                        out=mask[:64, :], in0=hist[:64, g * 512 : (g + 1) * 512],
                        scalar1=52.5, scalar2=None, op0=op.is_ge,
                    )
                    nc.tensor.matmul(
                        out=kps[:1, g * 512 : (g + 1) * 512],
                        lhsT=ones_bf[:64, :], rhs=mask[:64, :],
                        start=True, stop=True,
                    )
                nc.vector.tensor_reduce(
                    out=kv_sb[:1, c : c + 1], in_=kps[:],
                    axis=mybir.AxisListType.X, op=op.add,
                )

            nc.gpsimd.dma_start(out=kvals[:, :], in_=kv_sb[:])
    nc.compile()
    return nc


def build_fused(n_clouds=CLOUDS_PER_CORE, n_points=N, nchunk=NCHUNK, unroll=32):
    """Single-launch kernel: per cloud, coarse histogram -> threshold ->
    on-device candidate compaction -> fine refine -> K.  The candidate list
    never leaves the device (internal DRAM roundtrip broadcasts it across
    partitions); outputs are just kvals + ccount (a few bytes)."""
    import concourse.bass as bass
    import concourse.mybir as mybir
    from concourse.tile import TileContext

    f32, bf16 = mybir.dt.float32, mybir.dt.bfloat16
    i16, i32 = mybir.dt.int16, mybir.dt.int32
    op = mybir.AluOpType
    F = n_points // P
    cap = nchunk * P

    from concourse import bacc

    nc = bacc.Bacc("TRN2", target_bir_lowering=False, debug=False)
    qpk = nc.declare_dram_parameter(
        "qpk", [n_clouds, 2 * n_points + n_points // 2], mybir.dt.uint8, isOutput=False
    )
    kvals = nc.declare_dram_parameter("kvals", [1, n_clouds], f32, isOutput=True)
    ccount = nc.declare_dram_parameter("ccount", [1, n_clouds], f32, isOutput=True)
    candvs = nc.declare_dram_parameter("candvs", [n_clouds, cap], i16, isOutput=True)
    pscr = nc.dram_tensor("pscr", [P], f32)

    with TileContext(nc) as tc:
        with (
            tc.tile_pool(name="const", bufs=1) as constp,
            tc.tile_pool(name="work", bufs=2) as workp,
            tc.tile_pool(name="chain", bufs=1) as chainp,
            tc.tile_pool(name="hilo", bufs=2) as hilop,
            tc.tile_pool(name="oh", bufs=6) as ohp,
            tc.tile_pool(name="cmp", bufs=1) as cmpp,
            tc.tile_pool(name="row", bufs=1) as rowp,
            tc.tile_pool(name="sel", bufs=4) as selp,
            tc.tile_pool(name="mk", bufs=4) as mkp,
            tc.tile_pool(name="ps1", bufs=1, space="PSUM") as ps1p,
            tc.tile_pool(name="cps", bufs=1, space="PSUM") as cpsp,
            tc.tile_pool(name="ps2", bufs=1, space="PSUM") as ps2p,
        ):
            iota_i = constp.tile([P, P], i32)
            nc.gpsimd.iota(iota_i[:], pattern=[[1, P]], base=0, channel_multiplier=0)
            iota_bf = constp.tile([P, P], bf16)
            nc.vector.tensor_copy(out=iota_bf[:], in_=iota_i[:])
            iota_f = constp.tile([P, P], f32)
            nc.vector.tensor_copy(out=iota_f[:], in_=iota_i[:])
            iotacap_i = constp.tile([P, cap], i32)
            nc.gpsimd.iota(iotacap_i[:], pattern=[[1, cap]], base=0, channel_multiplier=0)
            iotacap_f = constp.tile([P, cap], f32)
            nc.vector.tensor_copy(out=iotacap_f[:], in_=iotacap_i[:])
            iota64_bf = constp.tile([P, 64], bf16)
            nc.vector.tensor_copy(out=iota64_bf[:], in_=iota_i[:, :64])
            ones_bf = constp.tile([P, 1], bf16)
            nc.vector.memset(ones_bf[:], 1.0)
            jge1 = constp.tile([1, cap], f32)
            nc.vector.tensor_scalar(
                out=jge1[:], in0=iotacap_f[0:1, :], scalar1=0.5, scalar2=None,
                op0=op.is_ge,
            )
            nc.gpsimd.dma_start(out=pscr[:], in_=iota_f[0:1, :])
            piota_f = constp.tile([P, 1], f32)
            nc.gpsimd.dma_start(out=piota_f[:], in_=pscr[:].rearrange("(b o) -> b o", o=1))
            p128_bf = constp.tile([P, 1], bf16)
            nc.vector.tensor_scalar(
                out=p128_bf[:], in0=piota_f[:], scalar1=128.0, scalar2=None, op0=op.mult
            )
            lt_bf = constp.tile([P, P], bf16)
            nc.vector.tensor_scalar(
                out=lt_bf[:], in0=iota_bf[:], scalar1=piota_f[:, 0:1], scalar2=None,
                op0=op.is_gt,
            )
            kv_sb = constp.tile([1, n_clouds], f32)
            cc_sb = constp.tile([1, n_clouds], f32)

            FC = min(512, F)
            for c in range(n_clouds):
                tc14 = workp.tile([P, F], f32, tag="tc14f")
                tlow6 = workp.tile([P, F], f32, tag="tlow6f")
                thi = hilop.tile([P, F], f32, tag="thi")
                tlo = hilop.tile([P, F], f32, tag="tlo")
                for col0 in range(0, F, FC):
                    sl = slice(col0, col0 + FC)
                    c14c, low6c, _ = _chain(
                        nc, tc, (workp, chainp), qpk, c, FC, mybir,
                        need_low6=True, col0=col0, n_points=n_points,
                    )
                    nc.vector.tensor_copy(out=tc14[:, sl], in_=c14c[:])
                    nc.vector.tensor_copy(out=tlow6[:, sl], in_=low6c[:])
                    thif = chainp.tile([P, FC], f32, tag="thif")
                    nc.vector.tensor_scalar(
                        out=thif[:], in0=c14c[:], scalar1=0.0078125,
                        scalar2=0.49609375, op0=op.mult, op1=op.subtract,
                    )
                    nc.vector.tensor_scalar(
                        out=thi[:, sl], in0=thif[:], scalar1=C23, scalar2=C23,
                        op0=op.add, op1=op.subtract,
                    )
                    nc.vector.scalar_tensor_tensor(
                        out=tlo[:, sl], in0=thi[:, sl], scalar=-128.0,
                        in1=c14c[:], op0=op.mult, op1=op.add,
                    )
                hist = ps1p.tile([P, P], f32, tag="hist")
                nc.vector.memset(hist[:], 0.0)

                def body1(iv, thi=thi, tlo=tlo, hist=hist):
                    ohh = ohp.tile([P, P], bf16, tag="ohh")
                    ohl = ohp.tile([P, P], bf16, tag="ohl")
                    nc.vector.tensor_scalar(
                        out=ohh[:], in0=iota_bf[:],
                        scalar1=thi[:, bass.ds(iv, 1)], scalar2=None, op0=op.is_equal,
                    )
                    nc.vector.tensor_scalar(
                        out=ohl[:], in0=iota_bf[:],
                        scalar1=tlo[:, bass.ds(iv, 1)], scalar2=None, op0=op.is_equal,
                    )
                    nc.tensor.matmul(
                        out=hist[:], lhsT=ohh[:], rhs=ohl[:],
                        start=False, stop=True, skip_group_check=True,
                    )

                tc.For_i_unrolled(0, F, 1, body1, max_unroll=unroll)

                # ---- threshold + compaction (see build_phase1) ----
                maskf = cmpp.tile([P, P], f32, tag="maskf")
                nc.vector.tensor_scalar(
                    out=maskf[:], in0=hist[:], scalar1=THRESH_COUNT - 0.5,
                    scalar2=None, op0=op.is_ge,
                )
                rc = cmpp.tile([P, 1], f32, tag="rc")
                nc.vector.tensor_reduce(
                    out=rc[:], in_=maskf[:], axis=mybir.AxisListType.X, op=op.add
                )
                rc_bf = cmpp.tile([P, 1], bf16, tag="rcbf")
                nc.vector.tensor_copy(out=rc_bf[:], in_=rc[:])
                pre_ps = cpsp.tile([P, 1], f32, tag="preps")
                nc.tensor.matmul(
                    out=pre_ps[:], lhsT=lt_bf[:], rhs=rc_bf[:], start=True, stop=True
                )
                pre_sb = cmpp.tile([P, 1], f32, tag="presb")
                nc.vector.tensor_copy(out=pre_sb[:], in_=pre_ps[:])
                sA = cmpp.tile([P, P], f32, tag="scanA")
                sB = cmpp.tile([P, P], f32, tag="scanB")
                cur, nxt = sA, sB
                nc.vector.tensor_copy(out=cur[:], in_=maskf[:])
                for sh in (1, 2, 4, 8, 16, 32, 64):
                    nc.vector.tensor_copy(out=nxt[:, :sh], in_=cur[:, :sh])
                    nc.vector.tensor_tensor(
                        out=nxt[:, sh:], in0=cur[:, sh:], in1=cur[:, : P - sh],
                        op=op.add,
                    )
                    cur, nxt = nxt, cur
                excl = cmpp.tile([P, P], f32, tag="excl")
                nc.vector.tensor_tensor(
                    out=excl[:], in0=cur[:], in1=maskf[:], op=op.subtract
                )
                slotA = cmpp.tile([P, P], f32, tag="slotA")
                nc.vector.tensor_scalar(
                    out=slotA[:], in0=excl[:], scalar1=pre_sb[:, 0:1],
                    scalar2=20000.0, op0=op.add, op1=op.subtract,
                )
                slotB = cmpp.tile([P, P], f32, tag="slotB")
                nc.vector.tensor_tensor(
                    out=slotB[:], in0=slotA[:], in1=maskf[:], op=op.mult
                )
                slot = cmpp.tile([P, P], f32, tag="slot")
                nc.vector.tensor_scalar(
                    out=slot[:], in0=slotB[:], scalar1=20000.0, scalar2=None, op0=op.add
                )
                cv_ps = cpsp.tile([1, cap], f32, tag="cvps")
                nc.vector.memset(cv_ps[:], 0.0)
                for l in range(P):
                    sel = selp.tile([P, cap], bf16, tag="sel")
                    nc.vector.tensor_scalar(
                        out=sel[:], in0=iotacap_f[:],
                        scalar1=slot[:, l : l + 1], scalar2=None, op0=op.is_equal,
                    )
                    for g in range(cap // 512):
                        gs = slice(g * 512, (g + 1) * 512)
                        nc.tensor.matmul(
                            out=cv_ps[:, gs], lhsT=p128_bf[:], rhs=sel[:, gs],
                            start=False, stop=True, skip_group_check=True,
                        )
                        nc.tensor.matmul(
                            out=cv_ps[:, gs], lhsT=iota_bf[:, l : l + 1], rhs=sel[:, gs],
                            start=False, stop=True, skip_group_check=True,
                        )
                # unused slots -> -1: cv==0 at j>=1 can only be an empty slot
                # (cell 0, the smallest id, always lands in slot 0 if present)
                zt = rowp.tile([1, cap], f32, tag="zt")
                nc.vector.tensor_scalar(
                    out=zt[:], in0=cv_ps[:], scalar1=0.5, scalar2=None, op0=op.is_lt
                )
                zz = rowp.tile([1, cap], f32, tag="zz")
                nc.vector.tensor_tensor(out=zz[:], in0=zt[:], in1=jge1[:], op=op.mult)
                cfin = rowp.tile([1, cap], f32, tag="cfin")
                nc.vector.tensor_tensor(
                    out=cfin[:], in0=cv_ps[:], in1=zz[:], op=op.subtract
                )
                cfin_i = rowp.tile([1, cap], i16, tag="cfini")
                nc.vector.tensor_copy(out=cfin_i[:], in_=cfin[:])
                cnz = rowp.tile([1, cap], f32, tag="cnz")
                nc.vector.tensor_scalar(
                    out=cnz[:], in0=cfin[:], scalar1=-0.5, scalar2=None, op0=op.is_ge
                )
                nc.vector.tensor_reduce(
                    out=cc_sb[0:1, c : c + 1], in_=cnz[:],
                    axis=mybir.AxisListType.X, op=op.add,
                )
                # broadcast the candidate row to all partitions via DRAM
                nc.gpsimd.dma_start(out=candvs[c : c + 1, :], in_=cfin_i[:])
                candbc = workp.tile([P, cap], i16, tag="candbc")
                cand_src = bass.AP(
                    tensor=candvs.tensor if hasattr(candvs, "tensor") else candvs,
                    offset=c * cap,
                    ap=[[0, P], [1, cap]],
                )
                nc.gpsimd.dma_start(out=candbc[:], in_=cand_src)

                # ---- fine refine (see build_phase2) ----
                hist2 = ps2p.tile([P, cap], f32, tag="hist2")
                nc.vector.memset(hist2[:], 0.0)

                def body2(iv, tc14=tc14, tlow6=tlow6, candbc=candbc, hist2=hist2):
                    memb = ohp.tile([P, cap], bf16, tag="memb")
                    loh = ohp.tile([P, 64], bf16, tag="loh")
                    nc.vector.tensor_scalar(
                        out=memb[:], in0=candbc[:],
                        scalar1=tc14[:, bass.ds(iv, 1)], scalar2=None, op0=op.is_equal,
                    )
                    nc.vector.tensor_scalar(
                        out=loh[:], in0=iota64_bf[:],
                        scalar1=tlow6[:, bass.ds(iv, 1)], scalar2=None, op0=op.is_equal,
                    )
                    for g in range(cap // 512):
                        nc.tensor.matmul(
                            out=hist2[:64, g * 512 : (g + 1) * 512],
                            lhsT=loh[:],
                            rhs=memb[:, g * 512 : (g + 1) * 512],
                            start=False, stop=True, skip_group_check=True,
                        )

                tc.For_i_unrolled(0, F, 1, body2, max_unroll=16)

                kps = cpsp.tile([1, cap], f32, tag="cvps")
                for g in range(cap // 512):
                    mask2 = mkp.tile([P, 512], bf16, tag="mask2")
                    nc.vector.tensor_scalar(
                        out=mask2[:64, :], in0=hist2[:64, g * 512 : (g + 1) * 512],
                        scalar1=52.5, scalar2=None, op0=op.is_ge,
                    )
                    nc.tensor.matmul(
                        out=kps[:1, g * 512 : (g + 1) * 512],
                        lhsT=ones_bf[:64, :], rhs=mask2[:64, :],
                        start=True, stop=True, skip_group_check=True,
                    )
                nc.vector.tensor_reduce(
                    out=kv_sb[:1, c : c + 1], in_=kps[:],
                    axis=mybir.AxisListType.X, op=op.add,
                )

            nc.gpsimd.dma_start(out=kvals[:, :], in_=kv_sb[:])
            nc.gpsimd.dma_start(out=ccount[:, :], in_=cc_sb[:])
    nc.compile()
    return nc


def _host_exact(points):
    """Exact numpy replica of the reference for one cloud. [N,2] f32 -> [TOPK]."""
    q = np.round(np.float32(1000.0) * points.astype(np.float32))
    xi = (q[:, 0] - q[:, 0].min()).astype(np.int64)
    zi = (q[:, 1] - q[:, 1].min()).astype(np.int64)
    idx = xi * GZ + zi
    counts = np.bincount(idx, minlength=1024 * GZ).astype(np.float32)
    occ = counts / np.float32(points.shape[0]) > np.float32(0.0002)
    k = min(int(occ.sum()), TOPK)
    out = np.zeros((TOPK,), np.float32)
    out[:k] = 1.0
    return out


def _numba_pack():
    if "nbpack" in _cache:
        return _cache["nbpack"]
    try:
        import numba

        @numba.njit(cache=False)
        def _nb_pack(pcd, out):
            nb, npts = pcd.shape[0], pcd.shape[1]
            qi = np.empty(2 * npts, np.int16)
            for b in range(nb):
                p = pcd[b].ravel()
                for i in range(2 * npts):
                    qi[i] = np.int16(np.rint(np.float32(1000.0) * p[i]))
                base = 2 * npts
                for i in range(2 * npts):
                    out[b, i] = qi[i] & 255
                for j in range(npts // 2):
                    h0 = ((qi[4 * j] >> 8) << 2) | (qi[4 * j + 1] >> 8)
                    h1 = ((qi[4 * j + 2] >> 8) << 2) | (qi[4 * j + 3] >> 8)
                    out[b, base + j] = np.uint8((h0 << 4) | h1)

        _cache["nbpack"] = _nb_pack
    except Exception:
        _cache["nbpack"] = None
    return _cache["nbpack"]


def _quantize_pack(pcd):
    """q = round_half_even(1000*pcd), bit-exact vs the reference's jnp.round
    (f32 multiply, then IEEE round-to-nearest-even), packed to 2.5 bytes per
    point: [2N u8 of interleaved low bytes | N/2 u8 of paired hi nibbles]."""
    nb = pcd.shape[0]
    npts = pcd.shape[1]
    out = np.empty((nb, 2 * npts + npts // 2), np.uint8)
    nbp = _numba_pack()
    if nbp is not None:
        nbp(pcd, out)
        return out
    t = np.empty((npts, 2), np.float32)
    for b in range(nb):
        np.multiply(pcd[b], np.float32(1000.0), out=t)
        np.rint(t, out=t)
        qi = t.astype(np.int16)  # [N, 2], values 0..1000
        v = qi.view(np.uint8)  # [N, 4]: qx_lo qx_hi qz_lo qz_hi
        lo = out[b, : 2 * npts].reshape(npts, 2)
        lo[:, 0] = v[:, 0]
        lo[:, 1] = v[:, 2]
        hb = (v[:, 1] << 2) | v[:, 3]
        out[b, 2 * npts :] = (hb[0::2] << 4) | hb[1::2]
    return out


def _make_exec(nc, n_cores, mesh):
    """Persistent jit(shard_map(bass_exec)) wrapper for a compiled Bass
    module: built once, reused every call (C++ fast-path dispatch after the
    first).  Mirrors concourse.bass2jax.run_bass_via_pjrt but accepts
    device-resident jax arrays so large inputs upload only once."""
    import jax
    import concourse.mybir as mybir
    from concourse import bass2jax
    from jax.sharding import PartitionSpec
    from jax.experimental.shard_map import shard_map

    bass2jax.install_neuronx_cc_hook()
    assert nc.dbg_addr is None and not nc.dbg_callbacks

    partition_name = nc.partition_id_tensor.name if nc.partition_id_tensor else None
    in_names, out_names, out_avals = [], [], []
    for alloc in nc.m.functions[0].allocations:
        if not isinstance(alloc, mybir.MemoryLocationSet):
            continue
        name = alloc.memorylocations[0].name
        if alloc.kind == "ExternalInput":
            if name != partition_name:
                in_names.append(name)
        elif alloc.kind == "ExternalOutput":
            out_names.append(name)
            out_avals.append(
                jax.core.ShapedArray(tuple(alloc.tensor_shape), mybir.dt.np(alloc.dtype))
            )
    n_params = len(in_names)
    all_names = in_names + out_names + ([partition_name] if partition_name else [])
    donate = tuple(range(n_params, n_params + len(out_names)))

    def _body(*args):
        operands = list(args)
        if partition_name is not None:
            operands.append(bass2jax.partition_id_tensor())
        return tuple(
            bass2jax._bass_exec_p.bind(
                *operands,
                out_avals=tuple(out_avals),
                in_names=tuple(all_names),
                out_names=tuple(out_names),
                lowering_input_output_aliases=(),
                sim_require_finite=True,
                sim_require_nnan=True,
                nc=nc,
            )
        )

    nio = n_params + len(out_names)
    sharded = jax.jit(
        shard_map(
            _body,
            mesh=mesh,
            in_specs=(PartitionSpec("core"),) * nio,
            out_specs=(PartitionSpec("core"),) * len(out_names),
            check_rep=False,
        ),
        donate_argnums=donate,
        keep_unused=True,
    )
    zero_shapes = [
        ((n_cores * a.shape[0], *a.shape[1:]), a.dtype) for a in out_avals
    ]
    return sharded, in_names, out_names, zero_shapes


def _get_rt():
    if "rt" in _cache:
        return _cache["rt"]
    import jax
    from jax.sharding import Mesh, PartitionSpec, NamedSharding

    devices = jax.devices()[:NCORES]
    assert len(devices) == NCORES
    mesh = Mesh(np.asarray(devices), ("core",))
    sharding = NamedSharding(mesh, PartitionSpec("core"))
    ncf = build_fused()
    execf = _make_exec(ncf, NCORES, mesh)
    _cache["rt"] = (sharding, execf)
    return _cache["rt"]


def kernel(pcd):
    import os
    import time
    import jax

    tlog = []
    t0 = time.time()
    timing = bool(os.environ.get("KTIME"))

    def mark(label, val=None):
        if timing:
            if val is not None:
                jax.block_until_ready(val)
            tlog.append((label, time.time() - t0))

    pcd = np.ascontiguousarray(np.asarray(pcd), dtype=np.float32)
    assert pcd.shape == (B, N, 2), pcd.shape
    sharding, (exf, inf, outf, zf) = _get_rt()
    mark("rt")

    q = _quantize_pack(pcd)  # [B, 2N + N/2] uint8
    mark("quantize")
    devq = jax.device_put(q, sharding)  # the one 40MB upload
    mark("upload", devq)

    assert inf == ["qpk"] and outf == ["kvals", "ccount", "candvs"]
    kv, cc, _candvs = exf(devq, np.zeros(*zf[0]), np.zeros(*zf[1]), np.zeros(*zf[2]))
    try:
        kv.copy_to_host_async()
        cc.copy_to_host_async()
    except Exception:
        pass
    mark("fused", kv)
    kv_np = np.asarray(kv).reshape(B)
    cc_np = np.asarray(cc).reshape(B)
    overflow = cc_np >= CAND_CAP
    mark("kv_fetch")
    if timing:
        print(
            "KTIME "
            + " ".join(f"{l}={dt - p:.3f}" for (l, dt), p in zip(tlog, [0.0] + [d for _, d in tlog[:-1]])),
            flush=True,
        )

    out = np.zeros((B, TOPK, 1), np.float32)
    iota = np.arange(TOPK)
    for b in range(B):
        if overflow[b]:
            out[b, :, 0] = _host_exact(pcd[b])
        else:
            out[b, :, 0] = (iota < kv_np[b]).astype(np.float32)
    return out
